# revision 1
# baseline (speedup 1.0000x reference)
"""Trainium2 Bass kernel for nn_LocalModel (6-encoder local-attention transformer).

Sharding: data-parallel over batch — B=8 batch elements, one per NeuronCore.
Each core runs the full 6-layer encoder stack + final projection for its
batch element entirely on-chip (all weights resident in SBUF), returning a
[6]-vector; the host gathers them into the [8, 6] output.

Attention uses the zero-masked-softmax identity: with out-of-window scores
set to 0 (not -inf), softmax over the full sequence satisfies
    out_i = (sum_{j in W} (e^{s_ij} - 1) v_j + sum_all v_j)
          / (sum_{j in W} (e^{s_ij} - 1) + S)
so only the 128-wide banded scores are ever computed. The "+sum_all v / +S"
terms enter the PSUM accumulation via a K=1 matmul against an augmented
V-total row (64 v-columns + a ones-column per head).
"""
import sys
import numpy as np

sys.path.insert(0, "/opt/trn_rl_repo")

B, S, D = 8, 1024, 512
H, Dh, W = 8, 64, 64
HD = 2048           # ffn hidden
C = 6               # classes
ENC = 6
EPS = 1e-5
P = 128
KO = D // P         # 4
HC = HD // P        # 16
SCALE = Dh ** -0.5

_CACHE = {}
LAST_EXEC_NS = None
LAST_RESULTS = None
TRACE = False


def _build(affine: bool):
    import concourse.bass as bass
    import concourse.tile as tile
    from concourse import bacc, mybir
    from concourse.masks import make_identity

    f32 = mybir.dt.float32
    f32r = mybir.dt.float32r
    bf16 = mybir.dt.bfloat16
    AF = mybir.ActivationFunctionType
    OP = mybir.AluOpType

    nc = bacc.Bacc()
    d = {}
    d['xT'] = nc.declare_dram_parameter("xT", [D, S], f32r, isOutput=False)
    for w in ("wqT", "wkT", "wvT"):
        d[w] = nc.declare_dram_parameter(w, [D, D], f32r, isOutput=False)
    for b_ in ("bq", "bk"):
        d[b_] = nc.declare_dram_parameter(b_, [D], f32, isOutput=False)
    d['bv'] = nc.declare_dram_parameter("bv", [D], f32, isOutput=False)
    d['fc1T'] = nc.declare_dram_parameter("fc1T", [D, HD], f32r, isOutput=False)
    d['fc1b'] = nc.declare_dram_parameter("fc1b", [HD], f32, isOutput=False)
    d['fc2T'] = nc.declare_dram_parameter("fc2T", [HD, D], f32r, isOutput=False)
    d['fc2b'] = nc.declare_dram_parameter("fc2b", [D], f32, isOutput=False)
    d['mask'] = nc.declare_dram_parameter("mask", [P, 384], f32, isOutput=False)
    d['owT'] = nc.declare_dram_parameter("owT", [C, D, S], f32, isOutput=False)
    if affine:
        d['lng'] = nc.declare_dram_parameter("lng", [D], f32, isOutput=False)
        d['lnb'] = nc.declare_dram_parameter("lnb", [D], f32, isOutput=False)
    out_d = nc.declare_dram_parameter("out", [1, C], f32, isOutput=True)

    def bcast_ap(dram_h, parts=P):
        # replicate a [N] dram vector across `parts` partitions
        a = dram_h[:]
        return bass.AP(tensor=a.tensor, offset=a.offset,
                       ap=[[0, parts]] + [list(x) for x in a.ap])

    from contextlib import ExitStack
    with tile.TileContext(nc) as tc, ExitStack() as ctx:
        wpool = ctx.enter_context(tc.tile_pool(name="wpool", bufs=1))
        big = ctx.enter_context(tc.tile_pool(name="big", bufs=2))
        qkp = ctx.enter_context(tc.tile_pool(name="qkp", bufs=1))
        vap = ctx.enter_context(tc.tile_pool(name="vap", bufs=1))
        atp = ctx.enter_context(tc.tile_pool(name="atp", bufs=1))
        hp = ctx.enter_context(tc.tile_pool(name="hp", bufs=1))
        pp = ctx.enter_context(tc.tile_pool(name="pp", bufs=5))
        tmp = ctx.enter_context(tc.tile_pool(name="tmp", bufs=3))
        small = ctx.enter_context(tc.tile_pool(name="small", bufs=4))
        psA = ctx.enter_context(tc.tile_pool(name="psA", bufs=2, space="PSUM"))
        psS = ctx.enter_context(tc.tile_pool(name="psS", bufs=3, space="PSUM"))
        psV = ctx.enter_context(tc.tile_pool(name="psV", bufs=2, space="PSUM"))
        psT = ctx.enter_context(tc.tile_pool(name="psT", bufs=1, space="PSUM"))

        # ---- persistent loads ----
        wq_sb = wpool.tile([P, KO, D], f32r, tag="wq")
        wk_sb = wpool.tile([P, KO, D], f32r, tag="wk")
        wv_sb = wpool.tile([P, KO, D], f32r, tag="wv")
        for sb, dr in ((wq_sb, d['wqT']), (wk_sb, d['wkT']), (wv_sb, d['wvT'])):
            nc.sync.dma_start(sb, dr.rearrange("(ko p) n -> p ko n", p=P))
        fc1_sb = wpool.tile([P, KO, HD], f32r, tag="fc1")
        nc.sync.dma_start(fc1_sb, d['fc1T'].rearrange("(ko p) n -> p ko n", p=P))
        fc2_sb = wpool.tile([P, HC, D], f32r, tag="fc2")
        nc.sync.dma_start(fc2_sb, d['fc2T'].rearrange("(hc p) n -> p hc n", p=P))
        bq_sb = wpool.tile([P, KO], f32, tag="bq")
        bk_sb = wpool.tile([P, KO], f32, tag="bk")
        nc.sync.dma_start(bq_sb, d['bq'].rearrange("(ko p) -> p ko", p=P))
        nc.sync.dma_start(bk_sb, d['bk'].rearrange("(ko p) -> p ko", p=P))
        bv_bc = wpool.tile([P, D], f32, tag="bv")
        nc.gpsimd.dma_start(out=bv_bc, in_=bcast_ap(d['bv']))
        fc1b_sb = wpool.tile([P, HC], f32, tag="fc1b")
        nc.sync.dma_start(fc1b_sb, d['fc1b'].rearrange("(hc p) -> p hc", p=P))
        fc2b_bc = wpool.tile([P, D], f32, tag="fc2b")
        nc.gpsimd.dma_start(out=fc2b_bc, in_=bcast_ap(d['fc2b']))
        mask_sb = wpool.tile([P, 384], f32, tag="mask")
        nc.sync.dma_start(mask_sb, d['mask'][:])
        if affine:
            g_bc = wpool.tile([P, D], f32, tag="g")
            b_bc = wpool.tile([P, D], f32, tag="b")
            nc.gpsimd.dma_start(out=g_bc, in_=bcast_ap(d['lng']))
            nc.gpsimd.dma_start(out=b_bc, in_=bcast_ap(d['lnb']))
        ident = wpool.tile([P, P], f32, tag="id")
        make_identity(nc, ident)
        ones_col = wpool.tile([P, 1], f32, tag="onc")
        nc.vector.memset(ones_col, 1.0)
        ones_row = wpool.tile([1, P], f32, tag="onr")
        nc.vector.memset(ones_row, 1.0)
        eps_sb = wpool.tile([P, 1], f32, tag="eps")
        nc.vector.memset(eps_sb, EPS)
        vtot_sb = wpool.tile([1, H * 65], f32, tag="vtot")
        vtb = wpool.tile([P, H * 65], f32, tag="vtb")
        bv1k = wpool.tile([1, D], f32, tag="bv1k")
        nc.scalar.mul(out=bv1k, in_=bv_bc[0:1, :], mul=float(S))

        xT = big.tile([P, KO, S], f32r, tag="big")
        nc.sync.dma_start(xT, d['xT'].rearrange("(ko p) n -> p ko n", p=P))

        def layer_norm_to(src_ap, out_tile):
            """LayerNorm src [P,512] -> out_tile [P,512] (token-major)."""
            st = small.tile([P, 6], f32, tag="st")
            mv = small.tile([P, 2], f32, tag="mv")
            nc.vector.bn_stats(out=st, in_=src_ap)
            nc.vector.bn_aggr(out=mv, in_=st)
            rstd = small.tile([P, 1], f32, tag="rs")
            nc.scalar.activation(out=rstd, in_=mv[:, 1:2], func=AF.Sqrt,
                                 bias=eps_sb[:, 0:1])
            nc.vector.reciprocal(out=rstd, in_=rstd)
            nc.vector.tensor_scalar(out=out_tile, in0=src_ap,
                                    scalar1=mv[:, 0:1], scalar2=rstd,
                                    op0=OP.subtract, op1=OP.mult)
            if affine:
                nc.vector.tensor_tensor(out=out_tile, in0=out_tile, in1=g_bc,
                                        op=OP.mult)
                nc.vector.tensor_tensor(out=out_tile, in0=out_tile, in1=b_bc,
                                        op=OP.add)

        def transpose_to(src_tile, dst_tile, tb):
            """src [P, 512] token-major block tb -> dst [P, KO, S] feature-major."""
            for dc in range(KO):
                pt = psA.tile([P, 512], f32, tag="pj")
                nc.tensor.transpose(pt[:, :P], src_tile[:, dc * P:(dc + 1) * P],
                                    ident)
                nc.scalar.copy(out=dst_tile[:, dc, tb * P:(tb + 1) * P],
                               in_=pt[:, :P])

        for L in range(ENC):
            # ---------- QKV projections ----------
            va = vap.tile([P, 8, H, 65], bf16, tag="va")
            nc.vector.memset(va[:, :, :, 64:65], 1.0)
            # V first (frees xT earliest), token-major
            for tb in range(8):
                pv = psA.tile([P, 512], f32, tag="pj")
                for ko in range(KO):
                    nc.tensor.matmul(
                        pv, lhsT=xT[:, ko, tb * P:(tb + 1) * P],
                        rhs=wv_sb[:, ko, :],
                        start=(ko == 0), stop=(ko == KO - 1))
                nc.vector.tensor_tensor(
                    out=va[:, tb, :, 0:64],
                    in0=pv.rearrange("p (h a) -> p h a", a=64),
                    in1=bv_bc.rearrange("p (h a) -> p h a", a=64),
                    op=OP.add)
            # q/k per d'-chunk tiles (bf16) so scores can start per head pair
            q_t, k_t = [], []
            for mc in range(KO):
                qm = qkp.tile([P, S], bf16, tag=f"q{mc}")
                km = qkp.tile([P, S], bf16, tag=f"k{mc}")
                q_t.append(qm)
                k_t.append(km)
                for half in range(2):
                    cs = slice(half * 512, (half + 1) * 512)
                    pq = psA.tile([P, 512], f32, tag="pj")
                    for ko in range(KO):
                        nc.tensor.matmul(
                            pq, lhsT=wq_sb[:, ko, mc * P:(mc + 1) * P],
                            rhs=xT[:, ko, cs],
                            start=(ko == 0), stop=(ko == KO - 1))
                    nc.scalar.activation(out=qm[:, cs], in_=pq,
                                         func=AF.Identity, bias=bq_sb[:, mc:mc + 1])
                    pk = psA.tile([P, 512], f32, tag="pj")
                    for ko in range(KO):
                        nc.tensor.matmul(
                            pk, lhsT=wk_sb[:, ko, mc * P:(mc + 1) * P],
                            rhs=xT[:, ko, cs],
                            start=(ko == 0), stop=(ko == KO - 1))
                    nc.scalar.activation(out=km[:, cs], in_=pk,
                                         func=AF.Identity, bias=bk_sb[:, mc:mc + 1])

            # ---------- V totals: (sum_t x) @ wvT + S*bv (exact, f32) ----------
            xs32 = small.tile([P, KO], f32, tag="xs")
            nc.vector.reduce_sum(out=xs32, in_=xT.bitcast(f32),
                                 axis=mybir.AxisListType.X)
            xsr = small.tile([P, KO], f32r, tag="xsr")
            nc.scalar.copy(out=xsr, in_=xs32)
            pvt = psT.tile([1, D], f32, tag="vt")
            for ko in range(KO):
                nc.tensor.matmul(pvt, lhsT=xsr[:, ko:ko + 1],
                                 rhs=wv_sb[:, ko, :],
                                 start=(ko == 0), stop=(ko == KO - 1))
            nc.vector.tensor_tensor(
                out=vtot_sb.rearrange("p (h a) -> p h a", a=65)[:, :, 0:64],
                in0=pvt.rearrange("p (h a) -> p h a", a=64),
                in1=bv1k.rearrange("p (h a) -> p h a", a=64), op=OP.add)
            nc.vector.memset(
                vtot_sb.rearrange("p (h a) -> p h a", a=65)[:, :, 64:65],
                float(S))
            # broadcast V-totals across partitions for the DVE normalize
            for g in range(2):
                pvb = psV.tile([P, 260], f32, tag="av")
                nc.tensor.matmul(pvb, lhsT=ones_row[0:1, :],
                                 rhs=vtot_sb[0:1, 260 * g:260 * (g + 1)],
                                 start=True, stop=True)
                nc.scalar.copy(out=vtb[:, 260 * g:260 * (g + 1)], in_=pvb)

            # ---------- attention ----------
            a_tok = atp.tile([P, 8, D], f32, tag="at")
            for h in range(H):
                hr = slice(64 * (h % 2), 64 * (h % 2) + 64)
                hko = h // 2
                pcs = {}
                for kc in range(8):
                    q0 = max(0, kc - 1) * P
                    q1 = min(8, kc + 2) * P
                    qw = q1 - q0
                    ps = psS.tile([P, 384], f32, tag="s")
                    nc.tensor.matmul(
                        ps[:, :qw],
                        lhsT=k_t[hko][hr, kc * P:(kc + 1) * P],
                        rhs=q_t[hko][hr, q0:q1],
                        start=True, stop=True)
                    if kc == 0:
                        mk = mask_sb[:, 128:384]
                    elif kc == 7:
                        mk = mask_sb[:, 0:256]
                    else:
                        mk = mask_sb[:, :]
                    # p = (e^{s*scale} - 1)*mask  ==  e*mask - mask
                    pc32 = pp.tile([P, 384], f32, tag="p32")
                    nc.scalar.activation(out=pc32[:, :qw], in_=ps[:, :qw],
                                         func=AF.Exp, scale=SCALE)
                    nc.vector.tensor_tensor(out=pc32[:, :qw], in0=pc32[:, :qw],
                                            in1=mk, op=OP.mult)
                    pc = pp.tile([P, 384], bf16, tag="p")
                    nc.vector.tensor_tensor(out=pc[:, :qw], in0=pc32[:, :qw],
                                            in1=mk, op=OP.subtract)
                    pcs[kc] = pc
                for qb in range(8):
                    pav_full = psV.tile([P, 260], f32, tag="av")
                    pav = pav_full[:, :65]
                    kcs = [kc for kc in (qb - 1, qb, qb + 1) if 0 <= kc < 8]
                    for i, kc in enumerate(kcs):
                        off = (qb - max(0, kc - 1)) * P
                        nc.tensor.matmul(pav, lhsT=pcs[kc][:, off:off + P],
                                         rhs=va[:, kc, h, :],
                                         start=(i == 0), stop=(i == len(kcs) - 1))
                    rc = small.tile([P, 1], f32, tag="rc")
                    nc.vector.tensor_scalar_add(out=rc, in0=pav[:, 64:65],
                                                scalar1=float(S))
                    nc.vector.reciprocal(out=rc, in_=rc)
                    asl = a_tok[:, qb, h * 64:(h + 1) * 64]
                    nc.vector.tensor_tensor(out=asl, in0=pav[:, 0:64],
                                            in1=vtb[:, h * 65:h * 65 + 64],
                                            op=OP.add)
                    nc.vector.tensor_scalar_mul(out=asl, in0=asl, scalar1=rc)

            # ---------- LN1 -> x1T (feature-major) ----------
            x1T = big.tile([P, KO, S], f32r, tag="big")
            for qb in range(8):
                xn = tmp.tile([P, D], f32, tag="xn")
                layer_norm_to(a_tok[:, qb, :], xn)
                transpose_to(xn, x1T, qb)

            # ---------- FFN + residual + LN2 -> next xT ----------
            xT_next = big.tile([P, KO, S], f32r, tag="big")
            for tq in range(4):
                qs = slice(tq * 256, (tq + 1) * 256)
                hts = []
                for hc in range(HC):
                    ph = psA.tile([P, 512], f32, tag="pj")
                    for ko in range(KO):
                        nc.tensor.matmul(
                            ph[:, :256],
                            lhsT=fc1_sb[:, ko, hc * P:(hc + 1) * P],
                            rhs=x1T[:, ko, qs],
                            start=(ko == 0), stop=(ko == KO - 1))
                    ht = hp.tile([P, 256], f32r, tag=f"h{hc}")
                    nc.scalar.activation(out=ht, in_=ph[:, :256], func=AF.Relu,
                                         bias=fc1b_sb[:, hc:hc + 1])
                    hts.append(ht)
                for tb2 in range(2):
                    tb = tq * 2 + tb2
                    pf = psA.tile([P, 512], f32, tag="pj")
                    for hc in range(HC):
                        nc.tensor.matmul(
                            pf, lhsT=hts[hc][:, tb2 * P:(tb2 + 1) * P],
                            rhs=fc2_sb[:, hc, :],
                            start=(hc == 0), stop=(hc == HC - 1))
                    pr = psA.tile([P, 512], f32, tag="pj")
                    for dc in range(KO):
                        nc.tensor.transpose(
                            pr[:, dc * P:(dc + 1) * P],
                            x1T[:, dc, tb * P:(tb + 1) * P].bitcast(f32), ident)
                    f = tmp.tile([P, D], f32, tag="xn")
                    nc.vector.tensor_tensor(out=f, in0=pf, in1=fc2b_bc, op=OP.add)
                    nc.vector.tensor_tensor(out=f, in0=f, in1=pr, op=OP.add)
                    xn2 = tmp.tile([P, D], f32, tag="xn")
                    layer_norm_to(f, xn2)
                    transpose_to(xn2, xT_next, tb)
            xT = xT_next

        # ---------- final projection: out[r] = sum(xT * owT[r]) ----------
        red = wpool.tile([P, C], f32, tag="red")
        for r in range(C):
            acc = tmp.tile([P, D], f32, tag="xn")
            for ko in range(KO):
                for half in range(2):
                    wt = tmp.tile([P, D], f32, tag="xn")
                    nc.sync.dma_start(
                        wt, d['owT'][r, ko * P:(ko + 1) * P,
                                     half * 512:(half + 1) * 512])
                    if ko == 0 and half == 0:
                        nc.vector.tensor_tensor(
                            out=acc, in0=xT[:, 0, 0:512].bitcast(f32),
                            in1=wt, op=OP.mult)
                    else:
                        mt = tmp.tile([P, D], f32, tag="xn")
                        nc.vector.tensor_tensor(
                            out=mt,
                            in0=xT[:, ko, half * 512:(half + 1) * 512].bitcast(f32),
                            in1=wt, op=OP.mult)
                        nc.vector.tensor_tensor(out=acc, in0=acc, in1=mt,
                                                op=OP.add)
            nc.vector.reduce_sum(out=red[:, r:r + 1], in_=acc,
                                 axis=mybir.AxisListType.X)
        pout = psT.tile([1, 260], f32, tag="vt")
        nc.tensor.matmul(pout[0:1, 0:C], lhsT=ones_col[:, 0:1], rhs=red,
                         start=True, stop=True)
        osb = wpool.tile([1, C], f32, tag="osb")
        nc.scalar.copy(out=osb, in_=pout[0:1, 0:C])
        nc.sync.dma_start(out_d[:], osb)

    nc.compile()
    return nc


def _prep(inputs):
    """Host-side input prep shared across cores. Returns (common, per_core, affine)."""
    emb = np.asarray(inputs['emb'], dtype=np.float32)
    idx = np.asarray(inputs['inputs'])
    pos = np.arange(S, dtype=np.float32)[:, None]
    div = np.exp(-np.log(10000.0) * np.arange(0, D, 2, dtype=np.float32) / D)
    ang = pos * div
    pe = np.zeros((S, D), dtype=np.float32)
    pe[:, 0::2] = np.sin(ang)
    pe[:, 1::2] = np.cos(ang)
    x0 = emb[idx] + pe[None]  # [B, S, D]

    jj = np.arange(P)[:, None]
    ccols = np.arange(384)[None, :]
    delta = 128 + jj - ccols
    mask = ((delta >= -W) & (delta < W)).astype(np.float32)
    mask = np.ascontiguousarray(mask)

    ln_g = np.asarray(inputs['ln_g'], dtype=np.float32)
    ln_b = np.asarray(inputs['ln_b'], dtype=np.float32)
    affine = not (np.all(ln_g == 1.0) and np.all(ln_b == 0.0))

    out_w = np.asarray(inputs['out_w'], dtype=np.float32)
    owT = np.ascontiguousarray(
        out_w.reshape(C, S, D).transpose(0, 2, 1))  # [C, D, S]

    common = {
        'wqT': np.ascontiguousarray(np.asarray(inputs['wq'], np.float32).T),
        'wkT': np.ascontiguousarray(np.asarray(inputs['wk'], np.float32).T),
        'wvT': np.ascontiguousarray(np.asarray(inputs['wv'], np.float32).T),
        'bq': np.ascontiguousarray(np.asarray(inputs['bq'], np.float32)),
        'bk': np.ascontiguousarray(np.asarray(inputs['bk'], np.float32)),
        'bv': np.ascontiguousarray(np.asarray(inputs['bv'], np.float32)),
        'fc1T': np.ascontiguousarray(np.asarray(inputs['fc1_w'], np.float32).T),
        'fc1b': np.ascontiguousarray(np.asarray(inputs['fc1_b'], np.float32)),
        'fc2T': np.ascontiguousarray(np.asarray(inputs['fc2_w'], np.float32).T),
        'fc2b': np.ascontiguousarray(np.asarray(inputs['fc2_b'], np.float32)),
        'mask': mask,
        'owT': owT,
    }
    if affine:
        common['lng'] = np.ascontiguousarray(ln_g)
        common['lnb'] = np.ascontiguousarray(ln_b)
    per_core = [
        {'xT': np.ascontiguousarray(x0[b].T.astype(np.float32))}
        for b in range(B)
    ]
    return common, per_core, affine


def kernel(**inputs):
    global LAST_EXEC_NS, LAST_RESULTS
    from concourse.bass_utils import run_bass_kernel_spmd

    common, per_core, affine = _prep(inputs)
    if affine not in _CACHE:
        _CACHE[affine] = _build(affine)
    nc = _CACHE[affine]

    in_maps = [dict(common, **pc) for pc in per_core]
    res = run_bass_kernel_spmd(nc, in_maps, list(range(B)), trace=TRACE)
    LAST_EXEC_NS = res.exec_time_ns
    LAST_RESULTS = res
    out = np.stack([res.results[b]["out"][0] for b in range(B)], axis=0)
    out = out + np.asarray(inputs['out_b'], np.float32)[None, :]
    return out.astype(np.float32)



# revision 18
# speedup vs baseline: 1.7556x; 1.7556x over previous
"""Trainium2 Bass kernel for nn_LocalModel (6-encoder local-attention transformer).

Sharding: data-parallel over batch - B=8 batch elements, one per NeuronCore.
Each core runs the full 6-layer encoder stack + final projection for its
batch element entirely on-chip (all weights resident in SBUF in bf16),
returning a [6]-vector; the host gathers them into the [8, 6] output.

Attention uses the zero-masked-softmax identity: with out-of-window scores
set to 0 (not -inf), softmax over the full sequence satisfies
    out_i = (sum_{j in W} (e^{s_ij} - 1) v_j + sum_all v_j)
          / (sum_{j in W} (e^{s_ij} - 1) + S)
The banded scores are computed qb-centric: key blocks B_j = [128j-64,
128j+64) (tokens padded by 64 zeros each side) against query cols
[128j-128, 128j+128), giving uniform triangular masks. The "-1" term is
folded into the PSUM accumulation via negative-mask matmuls against a
64-shifted V copy (va_shift), so the DVE only does exp-mask multiply.
"""
import sys
import numpy as np

sys.path.insert(0, "/opt/trn_rl_repo")

B, S, D = 8, 1024, 512
H, Dh, W = 8, 64, 64
HD = 2048           # ffn hidden
C = 6               # classes
ENC = 6
EPS = 1e-5
P = 128
KO = D // P         # 4
HC = HD // P        # 16
SCALE = Dh ** -0.5
XW = 64 + S + 64    # padded token width for x / k tiles (1152)
QW = 128 + S + 128  # padded token width for q tiles (1280)

_CACHE = {}
LAST_EXEC_NS = None
LAST_RESULTS = None
TRACE = False


def _build(affine: bool):
    import os
    STAGE = int(os.environ.get("KSTAGE", "9"))
    import concourse.bass as bass
    import concourse.tile as tile
    from concourse import bacc, mybir
    from concourse.masks import make_identity

    f32 = mybir.dt.float32
    bf16 = mybir.dt.bfloat16
    AF = mybir.ActivationFunctionType
    OP = mybir.AluOpType

    nc = bacc.Bacc()
    d = {}
    d['xT'] = nc.declare_dram_parameter("xT", [P, KO, XW], bf16, isOutput=False)
    for w in ("wqT", "wkT", "wvT"):
        d[w] = nc.declare_dram_parameter(w, [P, KO, D], bf16, isOutput=False)
    d['fc1T'] = nc.declare_dram_parameter("fc1T", [P, KO, HD], bf16, isOutput=False)
    d['fc2T'] = nc.declare_dram_parameter("fc2T", [P, HC, D], bf16, isOutput=False)
    d['owT'] = nc.declare_dram_parameter("owT", [P, C, 8, D], bf16, isOutput=False)
    d['bq'] = nc.declare_dram_parameter("bq", [P, KO], f32, isOutput=False)
    d['bk'] = nc.declare_dram_parameter("bk", [P, KO], f32, isOutput=False)
    d['bv'] = nc.declare_dram_parameter("bv", [D], f32, isOutput=False)
    d['bv1k'] = nc.declare_dram_parameter("bv1k", [1, D], f32, isOutput=False)
    d['fc1b'] = nc.declare_dram_parameter("fc1b", [P, HC], f32, isOutput=False)
    d['fc2b'] = nc.declare_dram_parameter("fc2b", [1, D], bf16, isOutput=False)
    # masks: [m_int 256 | m_e0 256 | m_e8 256 | ntri_lo 128 | ntri_lo_e 128
    #         | ntri_up 128 | ntri_up_e 128]  (bf16)
    d['mask'] = nc.declare_dram_parameter("mask", [P, 1280], bf16, isOutput=False)
    if affine:
        d['lng'] = nc.declare_dram_parameter("lng", [D], f32, isOutput=False)
        d['lnb'] = nc.declare_dram_parameter("lnb", [D], f32, isOutput=False)
    out_d = nc.declare_dram_parameter("out", [1, C], f32, isOutput=True)

    def bcast_ap(dram_h, parts=P):
        # replicate a [N] dram vector across `parts` partitions
        a = dram_h[:]
        return bass.AP(tensor=a.tensor, offset=a.offset,
                       ap=[[0, parts]] + [list(x) for x in a.ap])

    def rep_mid(ap2d, reps):
        # [P, N] -> [P, reps, N] with stride-0 middle axis
        return bass.AP(tensor=ap2d.tensor, offset=ap2d.offset,
                       ap=[list(ap2d.ap[0]), [0, reps], list(ap2d.ap[1])])

    def rep_last(ap2d, reps):
        # [P, N] -> [P, N, reps] with stride-0 last axis
        return bass.AP(tensor=ap2d.tensor, offset=ap2d.offset,
                       ap=[list(ap2d.ap[0]), list(ap2d.ap[1]), [0, reps]])

    from contextlib import ExitStack
    with tile.TileContext(nc) as tc, ExitStack() as ctx:
        wpool = ctx.enter_context(tc.tile_pool(name="wpool", bufs=1))
        bigx = ctx.enter_context(tc.tile_pool(name="bigx", bufs=1))
        qkp = ctx.enter_context(tc.tile_pool(name="qkp", bufs=1))
        vap = ctx.enter_context(tc.tile_pool(name="vap", bufs=1))
        pcp = ctx.enter_context(tc.tile_pool(name="pcp", bufs=3))
        atp = ctx.enter_context(tc.tile_pool(name="atp", bufs=2))
        xnp = ctx.enter_context(tc.tile_pool(name="xnp", bufs=1))
        htp = ctx.enter_context(tc.tile_pool(name="htp", bufs=1))
        xxp = ctx.enter_context(tc.tile_pool(name="xxp", bufs=2))
        tmp = ctx.enter_context(tc.tile_pool(name="tmp", bufs=3))
        small = ctx.enter_context(tc.tile_pool(name="small", bufs=4))
        psA = ctx.enter_context(tc.tile_pool(name="psA", bufs=2, space="PSUM"))
        psS = ctx.enter_context(tc.tile_pool(name="psS", bufs=3, space="PSUM"))
        psV = ctx.enter_context(tc.tile_pool(name="psV", bufs=2, space="PSUM"))

        # ---- persistent loads (host pre-arranged; all contiguous DMAs) ----
        wq_sb = wpool.tile([P, KO, D], bf16, tag="wq")
        wk_sb = wpool.tile([P, KO, D], bf16, tag="wk")
        wv_sb = wpool.tile([P, KO, D], bf16, tag="wv")
        fc1_sb = wpool.tile([P, KO, HD], bf16, tag="fc1")
        fc2_sb = wpool.tile([P, HC, D], bf16, tag="fc2")
        for sb, key in ((wv_sb, 'wvT'), (wq_sb, 'wqT'), (wk_sb, 'wkT'),
                        (fc1_sb, 'fc1T'), (fc2_sb, 'fc2T')):
            nc.sync.dma_start(sb, d[key][:])
        bq_sb = wpool.tile([P, KO], f32, tag="bq")
        bk_sb = wpool.tile([P, KO], f32, tag="bk")
        nc.sync.dma_start(bq_sb, d['bq'][:])
        nc.sync.dma_start(bk_sb, d['bk'][:])
        bv_bc = wpool.tile([P, D], f32, tag="bv")
        nc.gpsimd.dma_start(out=bv_bc, in_=bcast_ap(d['bv']))
        bv1k_sb = wpool.tile([1, D], f32, tag="bv1k")
        nc.sync.dma_start(bv1k_sb, d['bv1k'][:])
        fc1b_sb = wpool.tile([P, HC], f32, tag="fc1b")
        nc.sync.dma_start(fc1b_sb, d['fc1b'][:])
        fc2b_sb = wpool.tile([1, D], bf16, tag="fc2b")
        nc.sync.dma_start(fc2b_sb, d['fc2b'][:])
        mask_sb = wpool.tile([P, 1280], bf16, tag="mask")
        nc.sync.dma_start(mask_sb, d['mask'][:])
        ow_sb = wpool.tile([P, C, 8, D], bf16, tag="ow")
        nc.sync.dma_start(ow_sb, d['owT'][:])
        if affine:
            g_bc = wpool.tile([P, D], f32, tag="g")
            b_bc = wpool.tile([P, D], f32, tag="b")
            nc.gpsimd.dma_start(out=g_bc, in_=bcast_ap(d['lng']))
            nc.gpsimd.dma_start(out=b_bc, in_=bcast_ap(d['lnb']))

        ident = wpool.tile([P, P], bf16, tag="id")
        make_identity(nc, ident)
        ones_col = wpool.tile([P, 1], f32, tag="onc")
        nc.vector.memset(ones_col, 1.0)
        ones1 = wpool.tile([1, P], bf16, tag="on1")
        nc.vector.memset(ones1, 1.0)
        eps_sb = wpool.tile([P, 1], f32, tag="eps")
        nc.vector.memset(eps_sb, EPS)
        # V-totals row [1, 2*(4*65)]; ones-slots hold S (set once)
        vtot_sb = wpool.tile([1, 520], f32, tag="vtot")
        nc.vector.memset(
            vtot_sb.rearrange("p (g i a) -> p g i a", g=2, a=65)[:, :, :, 64:65],
            float(S))
        vtb = wpool.tile([P, 520], f32, tag="vtb")
        red = wpool.tile([P, C, 8], f32, tag="red")
        osb = wpool.tile([1, C], f32, tag="osb")
        scr = wpool.tile([P, D], bf16, tag="scr")

        # mask views
        m_int = mask_sb[:, 0:256]
        m_e0 = mask_sb[:, 256:512]
        m_e8 = mask_sb[:, 512:768]
        ntri_lo = mask_sb[:, 768:896]
        ntri_lo_e = mask_sb[:, 896:1024]
        ntri_up = mask_sb[:, 1024:1152]
        ntri_up_e = mask_sb[:, 1152:1280]

        # x ping-pong tiles (padded, feature-major)
        xA = bigx.tile([P, KO, XW], bf16, tag="xA")
        xB = bigx.tile([P, KO, XW], bf16, tag="xB")
        nc.gpsimd.memset(xB[:, :, 0:64], 0.0)
        nc.gpsimd.memset(xB[:, :, 64 + S:XW], 0.0)
        nc.sync.dma_start(xA, d['xT'][:])
        x1T = bigx.tile([P, KO, S], bf16, tag="x1T")

        q_pad = [qkp.tile([P, QW], bf16, tag=f"q{mc}", name=f"q{mc}")
                 for mc in range(KO)]
        k_pad = [qkp.tile([P, XW], bf16, tag=f"k{mc}", name=f"k{mc}")
                 for mc in range(KO)]
        for mc in range(KO):
            nc.gpsimd.memset(q_pad[mc][:, 0:128], 0.0)
            nc.gpsimd.memset(q_pad[mc][:, 128 + S:QW], 0.0)
            nc.gpsimd.memset(k_pad[mc][:, 0:64], 0.0)
            nc.gpsimd.memset(k_pad[mc][:, 64 + S:XW], 0.0)
        va = vap.tile([P, 9, 520], bf16, tag="va")
        nc.vector.memset(
            va.rearrange("p s (i a) -> p s i a", a=65)[:, :, :, 64:65], 1.0)

        def layer_norm_to(src_ap, out_tile):
            """LayerNorm src [P,512] -> out_tile [P,512]."""
            st = small.tile([P, 6], f32, tag="st")
            mv = small.tile([P, 2], f32, tag="mv")
            nc.vector.bn_stats(out=st, in_=src_ap)
            nc.vector.bn_aggr(out=mv, in_=st)
            rstd = small.tile([P, 1], f32, tag="rs")
            nc.scalar.activation(out=rstd, in_=mv[:, 1:2], func=AF.Sqrt,
                                 bias=eps_sb[:, 0:1])
            nc.vector.reciprocal(out=rstd, in_=rstd)
            nc.vector.tensor_scalar(out=out_tile, in0=src_ap,
                                    scalar1=mv[:, 0:1], scalar2=rstd,
                                    op0=OP.subtract, op1=OP.mult)
            if affine:
                nc.vector.tensor_tensor(out=out_tile, in0=out_tile, in1=g_bc,
                                        op=OP.mult)
                nc.vector.tensor_tensor(out=out_tile, in0=out_tile, in1=b_bc,
                                        op=OP.add)

        xT = xA
        for L in range(ENC):
            xN = xB if (L % 2 == 0) else xA
            last = (L == ENC - 1)

            # ---------- V totals: (sum_t x) @ wv + S*bv (f32) ----------
            xs32 = small.tile([P, KO], f32, tag="xs")
            nc.vector.reduce_sum(out=xs32, in_=xT[:, :, :],
                                 axis=mybir.AxisListType.X)
            xsr = small.tile([P, KO], bf16, tag="xsr")
            nc.scalar.copy(out=xsr, in_=xs32)
            pvt = psS.tile([P, 512], f32, tag="s")
            for ko in range(KO):
                nc.tensor.matmul(pvt[0:1, :], lhsT=xsr[:, ko:ko + 1],
                                 rhs=wv_sb[:, ko, :],
                                 start=(ko == 0), stop=(ko == KO - 1))
            nc.vector.tensor_tensor(
                out=vtot_sb.rearrange("p (g i a) -> p g i a", g=2, a=65)[:, :, :, 0:64],
                in0=pvt[0:1, :].rearrange("p (g i a) -> p g i a", g=2, a=64),
                in1=bv1k_sb.rearrange("p (g i a) -> p g i a", g=2, a=64),
                op=OP.add)
            nc.gpsimd.partition_broadcast(out_ap=vtb[:, :], in_ap=vtot_sb[0:1, :])

            # ---------- V projection, 64-shifted token blocks ----------
            for stb in range(9):
                pv = psA.tile([P, 512], f32, tag="pj")
                for ko in range(KO):
                    nc.tensor.matmul(
                        pv, lhsT=xT[:, ko, 128 * stb:128 * stb + 128],
                        rhs=wv_sb[:, ko, :],
                        start=(ko == 0), stop=(ko == KO - 1))
                nc.vector.tensor_tensor(
                    out=va.rearrange("p s (g i a) -> p s g i a",
                                     g=2, a=65)[:, stb, :, :, 0:64],
                    in0=pv.rearrange("p (g i a) -> p g i a", g=2, a=64),
                    in1=bv_bc.rearrange("p (g i a) -> p g i a", g=2, a=64),
                    op=OP.add)

            # ---------- Q/K projections (feature-major, bf16, padded) ----------
            for mc in range(KO if STAGE >= 2 else 0):
                for half in range(2):
                    cs = slice(64 + half * 512, 64 + (half + 1) * 512)
                    pq = psA.tile([P, 512], f32, tag="pj")
                    for ko in range(KO):
                        nc.tensor.matmul(
                            pq, lhsT=wq_sb[:, ko, mc * P:(mc + 1) * P],
                            rhs=xT[:, ko, cs],
                            start=(ko == 0), stop=(ko == KO - 1))
                    nc.scalar.activation(
                        out=q_pad[mc][:, 128 + half * 512:128 + (half + 1) * 512],
                        in_=pq, func=AF.Identity, bias=bq_sb[:, mc:mc + 1])
                    pk = psA.tile([P, 512], f32, tag="pj")
                    for ko in range(KO):
                        nc.tensor.matmul(
                            pk, lhsT=wk_sb[:, ko, mc * P:(mc + 1) * P],
                            rhs=xT[:, ko, cs],
                            start=(ko == 0), stop=(ko == KO - 1))
                    nc.scalar.activation(
                        out=k_pad[mc][:, cs],
                        in_=pk, func=AF.Identity, bias=bk_sb[:, mc:mc + 1])

            # ---------- scores + probs (qb-centric key blocks) ----------
            # pc tiles per head-pair hp: [P, 512] = two heads x 256 query cols
            pcs = {}   # (hp, j) -> tile
            a_cur = None

            def attn_unit(j):
                """Scores/exp/mask for key-block j, all 8 heads."""
                mk = m_int if 0 < j < 8 else (m_e0 if j == 0 else m_e8)
                for h in range(H):
                    hp, sub = h // 2, h % 2
                    hr = slice(64 * sub, 64 * sub + 64)
                    sc = psS.tile([P, 256], f32, tag="s")
                    nc.tensor.matmul(
                        sc, lhsT=k_pad[hp][hr, 128 * j:128 * j + 128],
                        rhs=q_pad[hp][hr, 128 * j:128 * j + 256],
                        start=True, stop=True)
                    pc = pcp.tile([P, 256], bf16, tag=f"pc{h}", name=f"pc{h}")
                    nc.scalar.activation(out=pc, in_=sc, func=AF.Exp, scale=SCALE)
                    nc.vector.tensor_tensor(out=pc, in0=pc, in1=mk, op=OP.mult)
                    pcs[(h, j)] = pc

            def attn_out(qb):
                """AV + normalize + LN1 + transpose for query block qb."""
                a_tok = atp.tile([P, D], f32, tag="at")
                for g in range(2):  # head group: heads 4g..4g+3
                    pav = psV.tile([P, 260], f32, tag="av")
                    nlo = ntri_lo_e if qb == 0 else ntri_lo
                    nup = ntri_up_e if qb == 7 else ntri_up
                    nc.tensor.matmul(pav, lhsT=nlo,
                                     rhs=va[:, qb, g * 260:(g + 1) * 260],
                                     start=True, stop=False)
                    nc.tensor.matmul(pav, lhsT=nup,
                                     rhs=va[:, qb + 1, g * 260:(g + 1) * 260],
                                     start=False, stop=False)
                    for i in range(4):
                        h = 4 * g + i
                        po = pav[:, i * 65:i * 65 + 65]
                        nc.tensor.matmul(
                            po, lhsT=pcs[(h, qb)][:, 128:256],
                            rhs=va[:, qb, g * 260 + i * 65:g * 260 + i * 65 + 65],
                            start=False, stop=False)
                        nc.tensor.matmul(
                            po, lhsT=pcs[(h, qb + 1)][:, 0:128],
                            rhs=va[:, qb + 1, g * 260 + i * 65:g * 260 + i * 65 + 65],
                            start=False, stop=(i == 3))
                    rc4 = small.tile([P, 4], f32, tag="rc")
                    nc.vector.tensor_scalar_add(
                        out=rc4.rearrange("p (i o) -> p i o", o=1),
                        in0=pav.rearrange("p (i a) -> p i a", a=65)[:, :, 64:65],
                        scalar1=float(S))
                    nc.vector.reciprocal(out=rc4, in_=rc4)
                    asl = a_tok[:, g * 256:(g + 1) * 256]
                    nc.vector.tensor_tensor(
                        out=asl.rearrange("p (i a) -> p i a", a=64),
                        in0=pav.rearrange("p (i a) -> p i a", a=65)[:, :, 0:64],
                        in1=vtb[:, g * 260:(g + 1) * 260].rearrange(
                            "p (i a) -> p i a", a=65)[:, :, 0:64],
                        op=OP.add)
                    nc.vector.tensor_tensor(
                        out=asl.rearrange("p (i a) -> p i a", a=64),
                        in0=asl.rearrange("p (i a) -> p i a", a=64),
                        in1=rep_last(rc4, 64), op=OP.mult)
                # LN1 -> xn (kept for residual) -> x1T feature-major
                xn = xnp.tile([P, D], bf16, tag=f"xn{qb}", name=f"xnt{qb}")
                layer_norm_to(a_tok, xn)
                pt = psA.tile([P, 512], bf16, tag="pt", bufs=1)
                for dc in range(KO):
                    nc.tensor.transpose(pt[:, dc * P:(dc + 1) * P],
                                        xn[:, dc * P:(dc + 1) * P], ident)
                nc.scalar.copy(
                    out=x1T[:, :, qb * P:(qb + 1) * P],
                    in_=pt.rearrange("p (ko t) -> p ko t", t=P))
                return xn

            if STAGE <= 2:
                continue
            xns = [None] * 8
            attn_unit(0)
            for j in range(1, 9):
                if STAGE >= 4:
                    attn_unit(j)
                if STAGE >= 5:
                    xns[j - 1] = attn_out(j - 1)
            if STAGE <= 5:
                continue

            # ---------- FFN + residual + LN2 ----------
            hts = [htp.tile([P, 512], bf16, tag=f"h{hc}", name=f"h{hc}")
                   for hc in range(HC)]
            for half in range(2):
                qs = slice(half * 512, (half + 1) * 512)
                for hc in range(HC):
                    ph = psA.tile([P, 512], f32, tag="pj")
                    for ko in range(KO):
                        nc.tensor.matmul(
                            ph, lhsT=fc1_sb[:, ko, hc * P:(hc + 1) * P],
                            rhs=x1T[:, ko, qs],
                            start=(ko == 0), stop=(ko == KO - 1))
                    nc.scalar.activation(out=hts[hc], in_=ph, func=AF.Relu,
                                         bias=fc1b_sb[:, hc:hc + 1])
                for tb2 in range(4):
                    tb = half * 4 + tb2
                    pf = psA.tile([P, 512], f32, tag="pj")
                    # fc2 bias via K=1 matmul (start=True zeroes the bank)
                    nc.tensor.matmul(pf, lhsT=ones1[0:1, :], rhs=fc2b_sb[0:1, :],
                                     start=True, stop=False)
                    for hc in range(HC):
                        nc.tensor.matmul(
                            pf, lhsT=hts[hc][:, tb2 * P:(tb2 + 1) * P],
                            rhs=fc2_sb[:, hc, :],
                            start=False, stop=False)
                    # residual via identity matmul
                    nc.tensor.matmul(pf, lhsT=ident, rhs=xns[tb],
                                     start=False, stop=True)
                    xo = xxp.tile([P, D], bf16, tag="xo")
                    layer_norm_to(pf, xo)
                    if not last:
                        pt = psA.tile([P, 512], bf16, tag="pt", bufs=1)
                        for dc in range(KO):
                            nc.tensor.transpose(pt[:, dc * P:(dc + 1) * P],
                                                xo[:, dc * P:(dc + 1) * P], ident)
                        nc.scalar.copy(
                            out=xN[:, :, 64 + tb * P:64 + (tb + 1) * P],
                            in_=pt.rearrange("p (ko t) -> p ko t", t=P))
                    else:
                        # final projection partial: red[p, c, tb]
                        for c in range(C):
                            nc.vector.tensor_tensor(
                                out=scr, in0=xo, in1=ow_sb[:, c, tb, :],
                                op=OP.mult)
                            nc.vector.reduce_sum(
                                out=red[:, c, tb:tb + 1], in_=scr,
                                axis=mybir.AxisListType.X)
            xT = xN

        # ---------- final cross-partition reduce ----------
        if STAGE <= 8:
            nc.vector.memset(red[:, :, :], 0.0)
        pout = psS.tile([P, 512], f32, tag="s")
        nc.tensor.matmul(pout[0:1, 0:C * 8], lhsT=ones_col[:, 0:1],
                         rhs=red.rearrange("p c t -> p (c t)"),
                         start=True, stop=True)
        nc.vector.reduce_sum(
            out=osb, in_=pout[0:1, 0:C * 8].rearrange("p (c t) -> p c t", t=8),
            axis=mybir.AxisListType.X)
        nc.sync.dma_start(out_d[:], osb)

    nc.compile()
    return nc


def _prep(inputs):
    """Host-side input prep shared across cores. Returns (common, per_core, affine)."""
    import ml_dtypes
    bf = ml_dtypes.bfloat16

    emb = np.asarray(inputs['emb'], dtype=np.float32)
    idx = np.asarray(inputs['inputs'])
    pos = np.arange(S, dtype=np.float32)[:, None]
    div = np.exp(-np.log(10000.0) * np.arange(0, D, 2, dtype=np.float32) / D)
    ang = pos * div
    pe = np.zeros((S, D), dtype=np.float32)
    pe[:, 0::2] = np.sin(ang)
    pe[:, 1::2] = np.cos(ang)
    x0 = emb[idx] + pe[None]  # [B, S, D]

    # masks (bf16): interior M[p,c] = (p < c) & (p >= c-128) for key block
    # B_j vs query cols [128j-128, 128j+128)
    p_ = np.arange(P)[:, None]
    c_ = np.arange(256)[None, :]
    m_int = ((p_ < c_) & (p_ >= c_ - 128)).astype(np.float32)
    m_e0 = m_int * (p_ >= 64)     # j=0: keys [-64, 64), first 64 partitions fake
    m_e8 = m_int * (p_ < 64)      # j=8: keys [960, 1088), last 64 fake
    c128 = np.arange(128)[None, :]
    tri_lo = (p_ >= c128).astype(np.float32)
    tri_up = (p_ < c128).astype(np.float32)
    ntri_lo = -tri_lo
    ntri_lo_e = -(tri_lo * (p_ >= 64))
    ntri_up = -tri_up
    ntri_up_e = -(tri_up * (p_ < 64))
    mask = np.concatenate(
        [m_int, m_e0, m_e8, ntri_lo, ntri_lo_e, ntri_up, ntri_up_e],
        axis=1).astype(bf)

    ln_g = np.asarray(inputs['ln_g'], dtype=np.float32)
    ln_b = np.asarray(inputs['ln_b'], dtype=np.float32)
    affine = not (np.all(ln_g == 1.0) and np.all(ln_b == 0.0))

    def wmaj(wT, ko):  # [D, N] feature-major -> [P, ko, N]
        N = wT.shape[1]
        return np.ascontiguousarray(
            wT.reshape(ko, P, N).transpose(1, 0, 2)).astype(bf)

    out_w = np.asarray(inputs['out_w'], dtype=np.float32)
    owT = np.ascontiguousarray(
        out_w.reshape(C, 8, P, D).transpose(2, 0, 1, 3)).astype(bf)  # [P,C,8,D]

    bq = np.asarray(inputs['bq'], np.float32)
    bk = np.asarray(inputs['bk'], np.float32)
    bv = np.asarray(inputs['bv'], np.float32)
    fc1b = np.asarray(inputs['fc1_b'], np.float32)

    common = {
        'wqT': wmaj(np.asarray(inputs['wq'], np.float32).T, KO),
        'wkT': wmaj(np.asarray(inputs['wk'], np.float32).T, KO),
        'wvT': wmaj(np.asarray(inputs['wv'], np.float32).T, KO),
        'fc1T': wmaj(np.asarray(inputs['fc1_w'], np.float32).T, KO),
        'fc2T': wmaj(np.asarray(inputs['fc2_w'], np.float32).T, HC),
        'bq': np.ascontiguousarray(bq.reshape(KO, P).T),
        'bk': np.ascontiguousarray(bk.reshape(KO, P).T),
        'bv': np.ascontiguousarray(bv),
        'bv1k': np.ascontiguousarray(bv[None, :] * float(S)),
        'fc1b': np.ascontiguousarray(fc1b.reshape(HC, P).T),
        'fc2b': np.ascontiguousarray(
            np.asarray(inputs['fc2_b'], np.float32)[None, :]).astype(bf),
        'mask': mask,
        'owT': owT,
    }
    if affine:
        common['lng'] = np.ascontiguousarray(ln_g)
        common['lnb'] = np.ascontiguousarray(ln_b)
    per_core = []
    for b in range(B):
        xp = np.zeros((D, XW), dtype=np.float32)
        xp[:, 64:64 + S] = x0[b].T
        per_core.append({'xT': np.ascontiguousarray(
            xp.reshape(KO, P, XW).transpose(1, 0, 2)).astype(bf)})
    return common, per_core, affine


def kernel(**inputs):
    global LAST_EXEC_NS, LAST_RESULTS
    from concourse.bass_utils import run_bass_kernel_spmd

    common, per_core, affine = _prep(inputs)
    if affine not in _CACHE:
        _CACHE[affine] = _build(affine)
    nc = _CACHE[affine]

    in_maps = [dict(common, **pc) for pc in per_core]
    res = run_bass_kernel_spmd(nc, in_maps, list(range(B)), trace=TRACE)
    LAST_EXEC_NS = res.exec_time_ns
    LAST_RESULTS = res
    out = np.stack([res.results[b]["out"][0] for b in range(B)], axis=0)
    out = out + np.asarray(inputs['out_b'], np.float32)[None, :]
    return out.astype(np.float32)


# revision 29
# speedup vs baseline: 1.7894x; 1.0193x over previous
"""Trainium2 Bass kernel for nn_LocalModel (6-encoder local-attention transformer).

Sharding: data-parallel over batch - B=8 batch elements, one per NeuronCore.
Each core runs the full 6-layer encoder stack + final projection for its
batch element entirely on-chip (all weights resident in SBUF in bf16),
returning a [6]-vector; the host gathers them into the [8, 6] output.

Attention uses the zero-masked-softmax identity: with out-of-window scores
set to 0 (not -inf), softmax over the full sequence satisfies
    out_i = (sum_{j in W} (e^{s_ij} - 1) v_j + sum_all v_j)
          / (sum_{j in W} (e^{s_ij} - 1) + S)
The banded scores are computed qb-centric: key blocks B_j = [128j-64,
128j+64) (tokens padded by 64 zeros each side) against query cols
[128j-128, 128j+128), giving uniform triangular masks. The "-1" term is
folded into the PSUM accumulation via negative-mask matmuls against a
64-shifted V copy (va_shift), so the DVE only does exp-mask multiply.
"""
import sys
import numpy as np

sys.path.insert(0, "/opt/trn_rl_repo")

B, S, D = 8, 1024, 512
H, Dh, W = 8, 64, 64
HD = 2048           # ffn hidden
C = 6               # classes
ENC = 6
EPS = 1e-5
P = 128
KO = D // P         # 4
HC = HD // P        # 16
SCALE = Dh ** -0.5
XW = 64 + S + 64    # padded token width for x / k tiles (1152)
QW = 128 + S + 128  # padded token width for q tiles (1280)

_CACHE = {}
LAST_EXEC_NS = None
LAST_RESULTS = None
TRACE = False


def _build(affine: bool):
    import os
    STAGE = int(os.environ.get("KSTAGE", "9"))
    import concourse.bass as bass
    import concourse.tile as tile
    from concourse import bacc, mybir
    from concourse.masks import make_identity

    f32 = mybir.dt.float32
    bf16 = mybir.dt.bfloat16
    AF = mybir.ActivationFunctionType
    OP = mybir.AluOpType

    nc = bacc.Bacc()
    d = {}
    d['xT'] = nc.declare_dram_parameter("xT", [P, KO, XW], bf16, isOutput=False)
    for w in ("wqT", "wkT", "wvT"):
        d[w] = nc.declare_dram_parameter(w, [P, KO, D], bf16, isOutput=False)
    d['fc1T'] = nc.declare_dram_parameter("fc1T", [P, KO, HD], bf16, isOutput=False)
    d['fc2T'] = nc.declare_dram_parameter("fc2T", [P, HC, D], bf16, isOutput=False)
    d['owT'] = nc.declare_dram_parameter("owT", [P, C, 8, D], bf16, isOutput=False)
    d['bq'] = nc.declare_dram_parameter("bq", [P, KO], f32, isOutput=False)
    d['bk'] = nc.declare_dram_parameter("bk", [P, KO], f32, isOutput=False)
    d['bv'] = nc.declare_dram_parameter("bv", [D], f32, isOutput=False)
    d['bv1k'] = nc.declare_dram_parameter("bv1k", [1, D], f32, isOutput=False)
    d['fc1b'] = nc.declare_dram_parameter("fc1b", [P, HC], f32, isOutput=False)
    d['fc2b'] = nc.declare_dram_parameter("fc2b", [1, D], bf16, isOutput=False)
    # masks: [m_int 256 | m_e0 256 | m_e8 256 | ntri_lo 128 | ntri_lo_e 128
    #         | ntri_up 128 | ntri_up_e 128]  (bf16)
    d['mask'] = nc.declare_dram_parameter("mask", [P, 1280], bf16, isOutput=False)
    if affine:
        d['lng'] = nc.declare_dram_parameter("lng", [D], f32, isOutput=False)
        d['lnb'] = nc.declare_dram_parameter("lnb", [D], f32, isOutput=False)
    out_d = nc.declare_dram_parameter("out", [1, C], f32, isOutput=True)

    def bcast_ap(dram_h, parts=P):
        # replicate a [N] dram vector across `parts` partitions
        a = dram_h[:]
        return bass.AP(tensor=a.tensor, offset=a.offset,
                       ap=[[0, parts]] + [list(x) for x in a.ap])

    def rep_mid(ap2d, reps):
        # [P, N] -> [P, reps, N] with stride-0 middle axis
        return bass.AP(tensor=ap2d.tensor, offset=ap2d.offset,
                       ap=[list(ap2d.ap[0]), [0, reps], list(ap2d.ap[1])])

    def rep_last(ap2d, reps):
        # [P, N] -> [P, N, reps] with stride-0 last axis
        return bass.AP(tensor=ap2d.tensor, offset=ap2d.offset,
                       ap=[list(ap2d.ap[0]), list(ap2d.ap[1]), [0, reps]])

    from contextlib import ExitStack
    with tile.TileContext(nc) as tc, ExitStack() as ctx:
        wpool = ctx.enter_context(tc.tile_pool(name="wpool", bufs=1))
        bigx = ctx.enter_context(tc.tile_pool(name="bigx", bufs=1))
        qkp = ctx.enter_context(tc.tile_pool(name="qkp", bufs=1))
        vap = ctx.enter_context(tc.tile_pool(name="vap", bufs=1))
        pcp = ctx.enter_context(tc.tile_pool(name="pcp", bufs=3))
        atp = ctx.enter_context(tc.tile_pool(name="atp", bufs=2))
        xnp = ctx.enter_context(tc.tile_pool(name="xnp", bufs=1))
        htp = ctx.enter_context(tc.tile_pool(name="htp", bufs=1))
        xxp = ctx.enter_context(tc.tile_pool(name="xxp", bufs=2))
        tmp = ctx.enter_context(tc.tile_pool(name="tmp", bufs=3))
        small = ctx.enter_context(tc.tile_pool(name="small", bufs=4))
        psA = ctx.enter_context(tc.tile_pool(name="psA", bufs=2, space="PSUM"))
        psS = ctx.enter_context(tc.tile_pool(name="psS", bufs=3, space="PSUM"))
        psV = ctx.enter_context(tc.tile_pool(name="psV", bufs=2, space="PSUM"))

        # ---- persistent loads (host pre-arranged; all contiguous DMAs) ----
        # xA first so layer-0 V can start immediately; ow last (layer-6 only)
        xA = bigx.tile([P, KO, XW], bf16, tag="xA")
        nc.sync.dma_start(xA, d['xT'][:])
        wq_sb = wpool.tile([P, KO, D], bf16, tag="wq")
        wk_sb = wpool.tile([P, KO, D], bf16, tag="wk")
        wv_sb = wpool.tile([P, KO, D], bf16, tag="wv")
        fc1_sb = wpool.tile([P, KO, HD], bf16, tag="fc1")
        fc2_sb = wpool.tile([P, HC, D], bf16, tag="fc2")
        for sb, key in ((wv_sb, 'wvT'), (wq_sb, 'wqT'), (wk_sb, 'wkT'),
                        (fc1_sb, 'fc1T'), (fc2_sb, 'fc2T')):
            nc.sync.dma_start(sb, d[key][:])
        bq_sb = wpool.tile([P, KO], f32, tag="bq")
        bk_sb = wpool.tile([P, KO], f32, tag="bk")
        nc.sync.dma_start(bq_sb, d['bq'][:])
        nc.sync.dma_start(bk_sb, d['bk'][:])
        bv_bc = wpool.tile([P, D], f32, tag="bv")
        nc.gpsimd.dma_start(out=bv_bc, in_=bcast_ap(d['bv']))
        bv1k_sb = wpool.tile([1, D], f32, tag="bv1k")
        nc.sync.dma_start(bv1k_sb, d['bv1k'][:])
        fc1b_sb = wpool.tile([P, HC], f32, tag="fc1b")
        nc.sync.dma_start(fc1b_sb, d['fc1b'][:])
        fc2b_sb = wpool.tile([1, D], bf16, tag="fc2b")
        nc.sync.dma_start(fc2b_sb, d['fc2b'][:])
        mask_sb = wpool.tile([P, 1280], bf16, tag="mask")
        nc.sync.dma_start(mask_sb, d['mask'][:])
        ow_sb = wpool.tile([P, C, 8, D], bf16, tag="ow")
        nc.gpsimd.dma_start(out=ow_sb, in_=d['owT'][:])
        if affine:
            g_bc = wpool.tile([P, D], f32, tag="g")
            b_bc = wpool.tile([P, D], f32, tag="b")
            nc.gpsimd.dma_start(out=g_bc, in_=bcast_ap(d['lng']))
            nc.gpsimd.dma_start(out=b_bc, in_=bcast_ap(d['lnb']))

        ident = wpool.tile([P, P], bf16, tag="id")
        make_identity(nc, ident)
        ones_col = wpool.tile([P, 1], f32, tag="onc")
        nc.vector.memset(ones_col, 1.0)
        ones1 = wpool.tile([1, P], bf16, tag="on1")
        nc.vector.memset(ones1, 1.0)
        eps_sb = wpool.tile([P, 1], f32, tag="eps")
        nc.vector.memset(eps_sb, EPS)
        # V-totals row [1, 2*(4*65)]; ones-slots hold S (set once)
        vtot_sb = wpool.tile([1, 520], f32, tag="vtot")
        nc.vector.memset(
            vtot_sb.rearrange("p (g i a) -> p g i a", g=2, a=65)[:, :, :, 64:65],
            float(S))
        vtb = wpool.tile([P, 520], f32, tag="vtb")
        red = wpool.tile([P, C, 8], f32, tag="red")
        osb = wpool.tile([1, C], f32, tag="osb")
        scr = wpool.tile([P, D], bf16, tag="scr")

        # mask views
        m_int = mask_sb[:, 0:256]
        m_e0 = mask_sb[:, 256:512]
        m_e8 = mask_sb[:, 512:768]
        ntri_lo = mask_sb[:, 768:896]
        ntri_lo_e = mask_sb[:, 896:1024]
        ntri_up = mask_sb[:, 1024:1152]
        ntri_up_e = mask_sb[:, 1152:1280]

        # x ping-pong tiles (padded, feature-major)
        xB = bigx.tile([P, KO, XW], bf16, tag="xB")
        nc.gpsimd.memset(xB[:, :, 0:64], 0.0)
        nc.gpsimd.memset(xB[:, :, 64 + S:XW], 0.0)
        x1T = bigx.tile([P, KO, S], bf16, tag="x1T")

        q_pad = [qkp.tile([P, QW], bf16, tag=f"q{mc}", name=f"q{mc}")
                 for mc in range(KO)]
        k_pad = [qkp.tile([P, XW], bf16, tag=f"k{mc}", name=f"k{mc}")
                 for mc in range(KO)]
        for mc in range(KO):
            nc.gpsimd.memset(q_pad[mc][:, 0:128], 0.0)
            nc.gpsimd.memset(q_pad[mc][:, 128 + S:QW], 0.0)
            nc.gpsimd.memset(k_pad[mc][:, 0:64], 0.0)
            nc.gpsimd.memset(k_pad[mc][:, 64 + S:XW], 0.0)
        va = vap.tile([P, 9, 520], bf16, tag="va")
        nc.vector.memset(
            va.rearrange("p s (i a) -> p s i a", a=65)[:, :, :, 64:65], 1.0)

        def layer_norm_to(src_ap, out_tile):
            """LayerNorm src [P,512] -> out_tile [P,512]."""
            st = small.tile([P, 6], f32, tag="st")
            mv = small.tile([P, 2], f32, tag="mv")
            nc.vector.bn_stats(out=st, in_=src_ap)
            nc.vector.bn_aggr(out=mv, in_=st)
            rstd = small.tile([P, 1], f32, tag="rs")
            nc.scalar.activation(out=rstd, in_=mv[:, 1:2], func=AF.Sqrt,
                                 bias=eps_sb[:, 0:1])
            nc.vector.reciprocal(out=rstd, in_=rstd)
            nc.vector.tensor_scalar(out=out_tile, in0=src_ap,
                                    scalar1=mv[:, 0:1], scalar2=rstd,
                                    op0=OP.subtract, op1=OP.mult)
            if affine:
                nc.vector.tensor_tensor(out=out_tile, in0=out_tile, in1=g_bc,
                                        op=OP.mult)
                nc.vector.tensor_tensor(out=out_tile, in0=out_tile, in1=b_bc,
                                        op=OP.add)

        xT = xA
        for L in range(ENC):
            xN = xB if (L % 2 == 0) else xA
            last = (L == ENC - 1)

            # ---------- V projection, 64-shifted token blocks ----------
            for stb in range(9):
                pv = psA.tile([P, 512], f32, tag="pj")
                for ko in range(KO):
                    nc.tensor.matmul(
                        pv, lhsT=xT[:, ko, 128 * stb:128 * stb + 128],
                        rhs=wv_sb[:, ko, :],
                        start=(ko == 0), stop=(ko == KO - 1))
                nc.vector.tensor_tensor(
                    out=va.rearrange("p s (g i a) -> p s g i a",
                                     g=2, a=65)[:, stb, :, :, 0:64],
                    in0=pv.rearrange("p (g i a) -> p g i a", g=2, a=64),
                    in1=bv_bc.rearrange("p (g i a) -> p g i a", g=2, a=64),
                    op=OP.add)

            # ---------- Q/K projections (feature-major, bf16, padded) ----------
            for mc in range(KO if STAGE >= 2 else 0):
                for half in range(2):
                    cs = slice(64 + half * 512, 64 + (half + 1) * 512)
                    pq = psA.tile([P, 512], f32, tag="pj")
                    for ko in range(KO):
                        nc.tensor.matmul(
                            pq, lhsT=wq_sb[:, ko, mc * P:(mc + 1) * P],
                            rhs=xT[:, ko, cs],
                            start=(ko == 0), stop=(ko == KO - 1))
                    nc.scalar.activation(
                        out=q_pad[mc][:, 128 + half * 512:128 + (half + 1) * 512],
                        in_=pq, func=AF.Identity, bias=bq_sb[:, mc:mc + 1])
                    pk = psA.tile([P, 512], f32, tag="pj")
                    for ko in range(KO):
                        nc.tensor.matmul(
                            pk, lhsT=wk_sb[:, ko, mc * P:(mc + 1) * P],
                            rhs=xT[:, ko, cs],
                            start=(ko == 0), stop=(ko == KO - 1))
                    nc.scalar.activation(
                        out=k_pad[mc][:, cs],
                        in_=pk, func=AF.Identity, bias=bk_sb[:, mc:mc + 1])

            # ---------- V totals: (sum_t x) @ wv + S*bv (f32) ----------
            xs32 = small.tile([P, KO], f32, tag="xs")
            nc.vector.reduce_sum(out=xs32, in_=xT[:, :, :],
                                 axis=mybir.AxisListType.X)
            xsr = small.tile([P, KO], bf16, tag="xsr")
            nc.scalar.copy(out=xsr, in_=xs32)
            for g in range(2):
                pvg = psV.tile([P, 260], f32, tag="av")
                for ko in range(KO):
                    nc.tensor.matmul(
                        pvg[0:1, 0:256], lhsT=xsr[:, ko:ko + 1],
                        rhs=wv_sb[:, ko, g * 256:(g + 1) * 256],
                        start=(ko == 0), stop=(ko == KO - 1))
                nc.vector.tensor_tensor(
                    out=vtot_sb.rearrange("p (g i a) -> p g i a",
                                          g=2, a=65)[:, g, :, 0:64],
                    in0=pvg[0:1, 0:256].rearrange("p (i a) -> p i a", a=64),
                    in1=bv1k_sb[:, g * 256:(g + 1) * 256].rearrange(
                        "p (i a) -> p i a", a=64),
                    op=OP.add)
            nc.gpsimd.partition_broadcast(out_ap=vtb[:, :], in_ap=vtot_sb[0:1, :])

            # ---------- scores + probs (qb-centric key blocks) ----------
            # pc tiles per head-pair hp: [P, 512] = two heads x 256 query cols
            pcs = {}   # (hp, j) -> tile
            a_cur = None

            def attn_unit(j):
                """Scores/exp/mask for key-block j, all 8 heads."""
                mk = m_int if 0 < j < 8 else (m_e0 if j == 0 else m_e8)
                for h in range(H):
                    hp, sub = h // 2, h % 2
                    hr = slice(64 * sub, 64 * sub + 64)
                    sc = psS.tile([P, 256], f32, tag="s")
                    nc.tensor.matmul(
                        sc, lhsT=k_pad[hp][hr, 128 * j:128 * j + 128],
                        rhs=q_pad[hp][hr, 128 * j:128 * j + 256],
                        start=True, stop=True)
                    pc = pcp.tile([P, 256], bf16, tag=f"pc{h}", name=f"pc{h}")
                    nc.scalar.activation(out=pc, in_=sc, func=AF.Exp, scale=SCALE)
                    nc.vector.tensor_tensor(out=pc, in0=pc, in1=mk, op=OP.mult)
                    pcs[(h, j)] = pc

            def attn_av(qb):
                """AV + normalize for query block qb -> a_tok."""
                a_tok = atp.tile([P, D], f32, tag="at")
                for g in range(2):  # head group: heads 4g..4g+3
                    pav = psV.tile([P, 260], f32, tag="av")
                    nlo = ntri_lo_e if qb == 0 else ntri_lo
                    nup = ntri_up_e if qb == 7 else ntri_up
                    nc.tensor.matmul(pav, lhsT=nlo,
                                     rhs=va[:, qb, g * 260:(g + 1) * 260],
                                     start=True, stop=False)
                    nc.tensor.matmul(pav, lhsT=nup,
                                     rhs=va[:, qb + 1, g * 260:(g + 1) * 260],
                                     start=False, stop=False)
                    for i in range(4):
                        h = 4 * g + i
                        po = pav[:, i * 65:i * 65 + 65]
                        nc.tensor.matmul(
                            po, lhsT=pcs[(h, qb)][:, 128:256],
                            rhs=va[:, qb, g * 260 + i * 65:g * 260 + i * 65 + 65],
                            start=False, stop=False)
                        nc.tensor.matmul(
                            po, lhsT=pcs[(h, qb + 1)][:, 0:128],
                            rhs=va[:, qb + 1, g * 260 + i * 65:g * 260 + i * 65 + 65],
                            start=False, stop=(i == 3))
                    rc4 = small.tile([P, 4], f32, tag="rc")
                    nc.vector.tensor_scalar_add(
                        out=rc4.rearrange("p (i o) -> p i o", o=1),
                        in0=pav.rearrange("p (i a) -> p i a", a=65)[:, :, 64:65],
                        scalar1=float(S))
                    nc.vector.reciprocal(out=rc4, in_=rc4)
                    asl = a_tok[:, g * 256:(g + 1) * 256]
                    nc.vector.tensor_tensor(
                        out=asl.rearrange("p (i a) -> p i a", a=64),
                        in0=pav.rearrange("p (i a) -> p i a", a=65)[:, :, 0:64],
                        in1=vtb[:, g * 260:(g + 1) * 260].rearrange(
                            "p (i a) -> p i a", a=65)[:, :, 0:64],
                        op=OP.add)
                    nc.vector.tensor_tensor(
                        out=asl.rearrange("p (i a) -> p i a", a=64),
                        in0=asl.rearrange("p (i a) -> p i a", a=64),
                        in1=rep_last(rc4, 64), op=OP.mult)
                return a_tok

            def attn_ln(qb, a_tok):
                """LN1 -> xn (kept for residual) -> x1T feature-major."""
                xn = xnp.tile([P, D], bf16, tag=f"xn{qb}", name=f"xnt{qb}")
                layer_norm_to(a_tok, xn)
                pt = psA.tile([P, 512], bf16, tag="pt", bufs=1)
                for dc in range(KO):
                    nc.tensor.transpose(pt[:, dc * P:(dc + 1) * P],
                                        xn[:, dc * P:(dc + 1) * P], ident)
                nc.scalar.copy(
                    out=x1T[:, :, qb * P:(qb + 1) * P],
                    in_=pt.rearrange("p (ko t) -> p ko t", t=P))
                return xn

            if STAGE <= 2:
                continue
            xns = [None] * 8
            atoks = [None] * 8
            attn_unit(0)
            if STAGE >= 4:
                attn_unit(1)
                if STAGE >= 5:
                    atoks[0] = attn_av(0)
                for j in range(2, 9):
                    attn_unit(j)
                    if STAGE >= 5:
                        atoks[j - 1] = attn_av(j - 1)
                        xns[j - 2] = attn_ln(j - 2, atoks[j - 2])
                if STAGE >= 5:
                    xns[7] = attn_ln(7, atoks[7])
            if STAGE <= 5:
                continue

            # ---------- FFN + residual + LN2 ----------
            hts = [htp.tile([P, 512], bf16, tag=f"h{hc}", name=f"h{hc}")
                   for hc in range(HC)]
            pend = None

            def emit_xpose(tb, xo):
                pt = psA.tile([P, 512], bf16, tag="pt", bufs=1)
                for dc in range(KO):
                    nc.tensor.transpose(pt[:, dc * P:(dc + 1) * P],
                                        xo[:, dc * P:(dc + 1) * P], ident)
                nc.scalar.copy(
                    out=xN[:, :, 64 + tb * P:64 + (tb + 1) * P],
                    in_=pt.rearrange("p (ko t) -> p ko t", t=P))

            for half in range(2):
                qs = slice(half * 512, (half + 1) * 512)
                for hc in range(HC):
                    ph = psA.tile([P, 512], f32, tag="pj")
                    for ko in range(KO):
                        nc.tensor.matmul(
                            ph, lhsT=fc1_sb[:, ko, hc * P:(hc + 1) * P],
                            rhs=x1T[:, ko, qs],
                            start=(ko == 0), stop=(ko == KO - 1))
                    nc.scalar.activation(out=hts[hc], in_=ph, func=AF.Relu,
                                         bias=fc1b_sb[:, hc:hc + 1])
                for tb2 in range(4):
                    tb = half * 4 + tb2
                    pf = psA.tile([P, 512], f32, tag="pj")
                    # fc2 bias via K=1 matmul (start=True zeroes the bank)
                    nc.tensor.matmul(pf, lhsT=ones1[0:1, :], rhs=fc2b_sb[0:1, :],
                                     start=True, stop=False)
                    for hc in range(HC):
                        nc.tensor.matmul(
                            pf, lhsT=hts[hc][:, tb2 * P:(tb2 + 1) * P],
                            rhs=fc2_sb[:, hc, :],
                            start=False, stop=False)
                    # residual via identity matmul
                    nc.tensor.matmul(pf, lhsT=ident, rhs=xns[tb],
                                     start=False, stop=True)
                    xo = xxp.tile([P, D], bf16, tag="xo")
                    layer_norm_to(pf, xo)
                    if last:
                        # final projection partial: red[p, c, tb]
                        for c in range(C):
                            nc.vector.tensor_tensor(
                                out=scr, in0=xo, in1=ow_sb[:, c, tb, :],
                                op=OP.mult)
                            nc.vector.reduce_sum(
                                out=red[:, c, tb:tb + 1], in_=scr,
                                axis=mybir.AxisListType.X)
                    else:
                        # defer transpose one tb so PE never waits on LN2
                        if pend is not None:
                            emit_xpose(*pend)
                        pend = (tb, xo)
            if pend is not None:
                emit_xpose(*pend)
            xT = xN

        # ---------- final cross-partition reduce ----------
        if STAGE <= 8:
            nc.vector.memset(red[:, :, :], 0.0)
        pout = psS.tile([P, 256], f32, tag="s")
        nc.tensor.matmul(pout[0:1, 0:C * 8], lhsT=ones_col[:, 0:1],
                         rhs=red.rearrange("p c t -> p (c t)"),
                         start=True, stop=True)
        nc.vector.reduce_sum(
            out=osb, in_=pout[0:1, 0:C * 8].rearrange("p (c t) -> p c t", t=8),
            axis=mybir.AxisListType.X)
        nc.sync.dma_start(out_d[:], osb)

    nc.compile()
    return nc


def _prep(inputs):
    """Host-side input prep shared across cores. Returns (common, per_core, affine)."""
    import ml_dtypes
    bf = ml_dtypes.bfloat16

    emb = np.asarray(inputs['emb'], dtype=np.float32)
    idx = np.asarray(inputs['inputs'])
    pos = np.arange(S, dtype=np.float32)[:, None]
    div = np.exp(-np.log(10000.0) * np.arange(0, D, 2, dtype=np.float32) / D)
    ang = pos * div
    pe = np.zeros((S, D), dtype=np.float32)
    pe[:, 0::2] = np.sin(ang)
    pe[:, 1::2] = np.cos(ang)
    x0 = emb[idx] + pe[None]  # [B, S, D]

    # masks (bf16): interior M[p,c] = (p < c) & (p >= c-128) for key block
    # B_j vs query cols [128j-128, 128j+128)
    p_ = np.arange(P)[:, None]
    c_ = np.arange(256)[None, :]
    m_int = ((p_ < c_) & (p_ >= c_ - 128)).astype(np.float32)
    m_e0 = m_int * (p_ >= 64)     # j=0: keys [-64, 64), first 64 partitions fake
    m_e8 = m_int * (p_ < 64)      # j=8: keys [960, 1088), last 64 fake
    c128 = np.arange(128)[None, :]
    tri_lo = (p_ >= c128).astype(np.float32)
    tri_up = (p_ < c128).astype(np.float32)
    ntri_lo = -tri_lo
    ntri_lo_e = -(tri_lo * (p_ >= 64))
    ntri_up = -tri_up
    ntri_up_e = -(tri_up * (p_ < 64))
    mask = np.concatenate(
        [m_int, m_e0, m_e8, ntri_lo, ntri_lo_e, ntri_up, ntri_up_e],
        axis=1).astype(bf)

    ln_g = np.asarray(inputs['ln_g'], dtype=np.float32)
    ln_b = np.asarray(inputs['ln_b'], dtype=np.float32)
    affine = not (np.all(ln_g == 1.0) and np.all(ln_b == 0.0))

    def wmaj(wT, ko):  # [D, N] feature-major -> [P, ko, N]
        N = wT.shape[1]
        return np.ascontiguousarray(
            wT.reshape(ko, P, N).transpose(1, 0, 2)).astype(bf)

    out_w = np.asarray(inputs['out_w'], dtype=np.float32)
    owT = np.ascontiguousarray(
        out_w.reshape(C, 8, P, D).transpose(2, 0, 1, 3)).astype(bf)  # [P,C,8,D]

    bq = np.asarray(inputs['bq'], np.float32)
    bk = np.asarray(inputs['bk'], np.float32)
    bv = np.asarray(inputs['bv'], np.float32)
    fc1b = np.asarray(inputs['fc1_b'], np.float32)

    common = {
        'wqT': wmaj(np.asarray(inputs['wq'], np.float32).T, KO),
        'wkT': wmaj(np.asarray(inputs['wk'], np.float32).T, KO),
        'wvT': wmaj(np.asarray(inputs['wv'], np.float32).T, KO),
        'fc1T': wmaj(np.asarray(inputs['fc1_w'], np.float32).T, KO),
        'fc2T': wmaj(np.asarray(inputs['fc2_w'], np.float32).T, HC),
        'bq': np.ascontiguousarray(bq.reshape(KO, P).T),
        'bk': np.ascontiguousarray(bk.reshape(KO, P).T),
        'bv': np.ascontiguousarray(bv),
        'bv1k': np.ascontiguousarray(bv[None, :] * float(S)),
        'fc1b': np.ascontiguousarray(fc1b.reshape(HC, P).T),
        'fc2b': np.ascontiguousarray(
            np.asarray(inputs['fc2_b'], np.float32)[None, :]).astype(bf),
        'mask': mask,
        'owT': owT,
    }
    if affine:
        common['lng'] = np.ascontiguousarray(ln_g)
        common['lnb'] = np.ascontiguousarray(ln_b)
    per_core = []
    for b in range(B):
        xp = np.zeros((D, XW), dtype=np.float32)
        xp[:, 64:64 + S] = x0[b].T
        per_core.append({'xT': np.ascontiguousarray(
            xp.reshape(KO, P, XW).transpose(1, 0, 2)).astype(bf)})
    return common, per_core, affine


def kernel(**inputs):
    global LAST_EXEC_NS, LAST_RESULTS
    from concourse.bass_utils import run_bass_kernel_spmd

    common, per_core, affine = _prep(inputs)
    if affine not in _CACHE:
        _CACHE[affine] = _build(affine)
    nc = _CACHE[affine]

    in_maps = [dict(common, **pc) for pc in per_core]
    res = run_bass_kernel_spmd(nc, in_maps, list(range(B)), trace=TRACE)
    LAST_EXEC_NS = res.exec_time_ns
    LAST_RESULTS = res
    out = np.stack([res.results[b]["out"][0] for b in range(B)], axis=0)
    out = out + np.asarray(inputs['out_b'], np.float32)[None, :]
    return out.astype(np.float32)


# revision 34
# speedup vs baseline: 1.8038x; 1.0081x over previous
"""Trainium2 Bass kernel for nn_LocalModel (6-encoder local-attention transformer).

Sharding: data-parallel over batch - B=8 batch elements, one per NeuronCore.
Each core runs the full 6-layer encoder stack + final projection for its
batch element entirely on-chip (all weights resident in SBUF in bf16),
returning a [6]-vector; the host gathers them into the [8, 6] output.

Attention uses the zero-masked-softmax identity: with out-of-window scores
set to 0 (not -inf), softmax over the full sequence satisfies
    out_i = (sum_{j in W} (e^{s_ij} - 1) v_j + sum_all v_j)
          / (sum_{j in W} (e^{s_ij} - 1) + S)
The banded scores are computed qb-centric: key blocks B_j = [128j-64,
128j+64) (tokens padded by 64 zeros each side) against query cols
[128j-128, 128j+128), giving uniform triangular masks. The "-1" term is
folded into the PSUM accumulation via negative-mask matmuls against a
64-shifted V copy (va_shift), so the DVE only does exp-mask multiply.
"""
import sys
import numpy as np

sys.path.insert(0, "/opt/trn_rl_repo")

B, S, D = 8, 1024, 512
H, Dh, W = 8, 64, 64
HD = 2048           # ffn hidden
C = 6               # classes
ENC = 6
EPS = 1e-5
P = 128
KO = D // P         # 4
HC = HD // P        # 16
SCALE = Dh ** -0.5
XW = 64 + S + 64    # padded token width for x / k tiles (1152)
QW = 128 + S + 128  # padded token width for q tiles (1280)

_CACHE = {}
LAST_EXEC_NS = None
LAST_RESULTS = None
TRACE = False


def _build(affine: bool):
    import os
    STAGE = int(os.environ.get("KSTAGE", "9"))
    import concourse.bass as bass
    import concourse.tile as tile
    from concourse import bacc, mybir
    from concourse.masks import make_identity

    f32 = mybir.dt.float32
    bf16 = mybir.dt.bfloat16
    AF = mybir.ActivationFunctionType
    OP = mybir.AluOpType

    nc = bacc.Bacc()
    d = {}
    d['xT'] = nc.declare_dram_parameter("xT", [P, KO, XW], bf16, isOutput=False)
    for w in ("wqT", "wkT", "wvT"):
        d[w] = nc.declare_dram_parameter(w, [P, KO, D], bf16, isOutput=False)
    d['fc1T'] = nc.declare_dram_parameter("fc1T", [P, KO, HD], bf16, isOutput=False)
    d['fc2T'] = nc.declare_dram_parameter("fc2T", [P, HC, D], bf16, isOutput=False)
    d['owT'] = nc.declare_dram_parameter("owT", [P, C, 8, D], bf16, isOutput=False)
    d['bq'] = nc.declare_dram_parameter("bq", [P, KO], f32, isOutput=False)
    d['bk'] = nc.declare_dram_parameter("bk", [P, KO], f32, isOutput=False)
    d['bv'] = nc.declare_dram_parameter("bv", [D], f32, isOutput=False)
    d['bv1k'] = nc.declare_dram_parameter("bv1k", [1, D], f32, isOutput=False)
    d['fc1b'] = nc.declare_dram_parameter("fc1b", [P, HC], f32, isOutput=False)
    d['fc2b'] = nc.declare_dram_parameter("fc2b", [1, D], bf16, isOutput=False)
    # masks: [m_int 256 | m_e0 256 | m_e8 256 | ntri_lo 128 | ntri_lo_e 128
    #         | ntri_up 128 | ntri_up_e 128]  (bf16)
    d['mask'] = nc.declare_dram_parameter("mask", [P, 1280], bf16, isOutput=False)
    if affine:
        d['lng'] = nc.declare_dram_parameter("lng", [D], f32, isOutput=False)
        d['lnb'] = nc.declare_dram_parameter("lnb", [D], f32, isOutput=False)
    out_d = nc.declare_dram_parameter("out", [1, C], f32, isOutput=True)

    def bcast_ap(dram_h, parts=P):
        # replicate a [N] dram vector across `parts` partitions
        a = dram_h[:]
        return bass.AP(tensor=a.tensor, offset=a.offset,
                       ap=[[0, parts]] + [list(x) for x in a.ap])

    def rep_mid(ap2d, reps):
        # [P, N] -> [P, reps, N] with stride-0 middle axis
        return bass.AP(tensor=ap2d.tensor, offset=ap2d.offset,
                       ap=[list(ap2d.ap[0]), [0, reps], list(ap2d.ap[1])])

    def rep_last(ap2d, reps):
        # [P, N] -> [P, N, reps] with stride-0 last axis
        return bass.AP(tensor=ap2d.tensor, offset=ap2d.offset,
                       ap=[list(ap2d.ap[0]), list(ap2d.ap[1]), [0, reps]])

    from contextlib import ExitStack
    with tile.TileContext(nc) as tc, ExitStack() as ctx:
        wpool = ctx.enter_context(tc.tile_pool(name="wpool", bufs=1))
        bigx = ctx.enter_context(tc.tile_pool(name="bigx", bufs=1))
        qkp = ctx.enter_context(tc.tile_pool(name="qkp", bufs=1))
        vap = ctx.enter_context(tc.tile_pool(name="vap", bufs=1))
        pcp = ctx.enter_context(tc.tile_pool(name="pcp", bufs=3))
        atp = ctx.enter_context(tc.tile_pool(name="atp", bufs=2))
        xnp = ctx.enter_context(tc.tile_pool(name="xnp", bufs=1))
        htp = ctx.enter_context(tc.tile_pool(name="htp", bufs=1))
        xxp = ctx.enter_context(tc.tile_pool(name="xxp", bufs=2))
        tmp = ctx.enter_context(tc.tile_pool(name="tmp", bufs=3))
        small = ctx.enter_context(tc.tile_pool(name="small", bufs=4))
        psA = ctx.enter_context(tc.tile_pool(name="psA", bufs=2, space="PSUM"))
        psS = ctx.enter_context(tc.tile_pool(name="psS", bufs=3, space="PSUM"))
        psV = ctx.enter_context(tc.tile_pool(name="psV", bufs=2, space="PSUM"))

        # ---- persistent loads (host pre-arranged; all contiguous DMAs) ----
        # xA first so layer-0 V can start immediately; ow last (layer-6 only)
        xA = bigx.tile([P, KO, XW], bf16, tag="xA")
        nc.sync.dma_start(xA, d['xT'][:])
        wq_sb = wpool.tile([P, KO, D], bf16, tag="wq")
        wk_sb = wpool.tile([P, KO, D], bf16, tag="wk")
        wv_sb = wpool.tile([P, KO, D], bf16, tag="wv")
        fc1_sb = wpool.tile([P, KO, HD], bf16, tag="fc1")
        fc2_sb = wpool.tile([P, HC, D], bf16, tag="fc2")
        for sb, key in ((wv_sb, 'wvT'), (wq_sb, 'wqT'), (wk_sb, 'wkT'),
                        (fc1_sb, 'fc1T'), (fc2_sb, 'fc2T')):
            nc.sync.dma_start(sb, d[key][:])
        bq_sb = wpool.tile([P, KO], f32, tag="bq")
        bk_sb = wpool.tile([P, KO], f32, tag="bk")
        nc.sync.dma_start(bq_sb, d['bq'][:])
        nc.sync.dma_start(bk_sb, d['bk'][:])
        bv_bc = wpool.tile([P, D], f32, tag="bv")
        nc.gpsimd.dma_start(out=bv_bc, in_=bcast_ap(d['bv']))
        bv1k_sb = wpool.tile([1, D], f32, tag="bv1k")
        nc.sync.dma_start(bv1k_sb, d['bv1k'][:])
        fc1b_sb = wpool.tile([P, HC], f32, tag="fc1b")
        nc.sync.dma_start(fc1b_sb, d['fc1b'][:])
        fc2b_sb = wpool.tile([1, D], bf16, tag="fc2b")
        nc.sync.dma_start(fc2b_sb, d['fc2b'][:])
        mask_sb = wpool.tile([P, 1280], bf16, tag="mask")
        nc.sync.dma_start(mask_sb, d['mask'][:])
        ow_sb = wpool.tile([P, C, 8, D], bf16, tag="ow")
        nc.gpsimd.dma_start(out=ow_sb, in_=d['owT'][:])
        if affine:
            g_bc = wpool.tile([P, D], f32, tag="g")
            b_bc = wpool.tile([P, D], f32, tag="b")
            nc.gpsimd.dma_start(out=g_bc, in_=bcast_ap(d['lng']))
            nc.gpsimd.dma_start(out=b_bc, in_=bcast_ap(d['lnb']))

        ident = wpool.tile([P, P], bf16, tag="id")
        make_identity(nc, ident)
        ones_col = wpool.tile([P, 1], f32, tag="onc")
        nc.vector.memset(ones_col, 1.0)
        ones1 = wpool.tile([1, P], bf16, tag="on1")
        nc.vector.memset(ones1, 1.0)
        eps_sb = wpool.tile([P, 1], f32, tag="eps")
        nc.vector.memset(eps_sb, EPS)
        # V-totals row [1, 2*(4*65)]; ones-slots hold S (set once)
        vtot_sb = wpool.tile([1, 520], f32, tag="vtot")
        nc.vector.memset(
            vtot_sb.rearrange("p (g i a) -> p g i a", g=2, a=65)[:, :, :, 64:65],
            float(S))
        vtb = wpool.tile([P, 520], f32, tag="vtb")
        red = wpool.tile([P, C, 8], f32, tag="red")
        osb = wpool.tile([1, C], f32, tag="osb")
        scr = wpool.tile([P, D], bf16, tag="scr")
        scr2 = wpool.tile([P, D], bf16, tag="scr2")

        # mask views
        m_int = mask_sb[:, 0:256]
        m_e0 = mask_sb[:, 256:512]
        m_e8 = mask_sb[:, 512:768]
        ntri_lo = mask_sb[:, 768:896]
        ntri_lo_e = mask_sb[:, 896:1024]
        ntri_up = mask_sb[:, 1024:1152]
        ntri_up_e = mask_sb[:, 1152:1280]

        # x ping-pong tiles (padded, feature-major)
        xB = bigx.tile([P, KO, XW], bf16, tag="xB")
        nc.gpsimd.memset(xB[:, :, 0:64], 0.0)
        nc.gpsimd.memset(xB[:, :, 64 + S:XW], 0.0)
        x1T = bigx.tile([P, KO, S], bf16, tag="x1T")

        q_pad = [qkp.tile([P, QW], bf16, tag=f"q{mc}", name=f"q{mc}")
                 for mc in range(KO)]
        k_pad = [qkp.tile([P, XW], bf16, tag=f"k{mc}", name=f"k{mc}")
                 for mc in range(KO)]
        for mc in range(KO):
            nc.gpsimd.memset(q_pad[mc][:, 0:128], 0.0)
            nc.gpsimd.memset(q_pad[mc][:, 128 + S:QW], 0.0)
            nc.gpsimd.memset(k_pad[mc][:, 0:64], 0.0)
            nc.gpsimd.memset(k_pad[mc][:, 64 + S:XW], 0.0)
        va = vap.tile([P, 9, 520], bf16, tag="va")
        nc.vector.memset(
            va.rearrange("p s (i a) -> p s i a", a=65)[:, :, :, 64:65], 1.0)

        def layer_norm_to(src_ap, out_tile):
            """LayerNorm src [P,512] -> out_tile [P,512]."""
            st = small.tile([P, 6], f32, tag="st")
            mv = small.tile([P, 2], f32, tag="mv")
            nc.vector.bn_stats(out=st, in_=src_ap)
            nc.vector.bn_aggr(out=mv, in_=st)
            rstd = small.tile([P, 1], f32, tag="rs")
            nc.scalar.activation(out=rstd, in_=mv[:, 1:2], func=AF.Sqrt,
                                 bias=eps_sb[:, 0:1])
            nc.vector.reciprocal(out=rstd, in_=rstd)
            nc.vector.tensor_scalar(out=out_tile, in0=src_ap,
                                    scalar1=mv[:, 0:1], scalar2=rstd,
                                    op0=OP.subtract, op1=OP.mult)
            if affine:
                nc.vector.tensor_tensor(out=out_tile, in0=out_tile, in1=g_bc,
                                        op=OP.mult)
                nc.vector.tensor_tensor(out=out_tile, in0=out_tile, in1=b_bc,
                                        op=OP.add)

        xT = xA
        for L in range(ENC):
            xN = xB if (L % 2 == 0) else xA
            last = (L == ENC - 1)

            # ---------- V projection, 64-shifted token blocks ----------
            # xs reduce for V-totals is split per-ko and interleaved so the
            # DVE never blocks the V bias-adds for long.
            xs32 = small.tile([P, KO], f32, tag="xs")
            for stb in range(9):
                pv = psA.tile([P, 512], f32, tag="pj")
                for ko in range(KO):
                    nc.tensor.matmul(
                        pv, lhsT=xT[:, ko, 128 * stb:128 * stb + 128],
                        rhs=wv_sb[:, ko, :],
                        start=(ko == 0), stop=(ko == KO - 1))
                nc.vector.tensor_tensor(
                    out=va.rearrange("p s (g i a) -> p s g i a",
                                     g=2, a=65)[:, stb, :, :, 0:64],
                    in0=pv.rearrange("p (g i a) -> p g i a", g=2, a=64),
                    in1=bv_bc.rearrange("p (g i a) -> p g i a", g=2, a=64),
                    op=OP.add)
                if stb < KO:
                    nc.vector.reduce_sum(out=xs32[:, stb:stb + 1],
                                         in_=xT[:, stb:stb + 1, :],
                                         axis=mybir.AxisListType.X)

            # ---------- Q/K projections (feature-major, bf16, padded) ----------
            for mc in range(KO if STAGE >= 2 else 0):
                for half in range(2):
                    cs = slice(64 + half * 512, 64 + (half + 1) * 512)
                    pq = psA.tile([P, 512], f32, tag="pj")
                    for ko in range(KO):
                        nc.tensor.matmul(
                            pq, lhsT=wq_sb[:, ko, mc * P:(mc + 1) * P],
                            rhs=xT[:, ko, cs],
                            start=(ko == 0), stop=(ko == KO - 1))
                    nc.scalar.activation(
                        out=q_pad[mc][:, 128 + half * 512:128 + (half + 1) * 512],
                        in_=pq, func=AF.Identity, bias=bq_sb[:, mc:mc + 1])
                    pk = psA.tile([P, 512], f32, tag="pj")
                    for ko in range(KO):
                        nc.tensor.matmul(
                            pk, lhsT=wk_sb[:, ko, mc * P:(mc + 1) * P],
                            rhs=xT[:, ko, cs],
                            start=(ko == 0), stop=(ko == KO - 1))
                    nc.scalar.activation(
                        out=k_pad[mc][:, cs],
                        in_=pk, func=AF.Identity, bias=bk_sb[:, mc:mc + 1])

            # ---------- V totals: (sum_t x) @ wv + S*bv (f32) ----------
            xsr = small.tile([P, KO], bf16, tag="xsr")
            nc.scalar.copy(out=xsr, in_=xs32)
            for g in range(2):
                pvg = psV.tile([P, 260], f32, tag="av")
                for ko in range(KO):
                    nc.tensor.matmul(
                        pvg[0:1, 0:256], lhsT=xsr[:, ko:ko + 1],
                        rhs=wv_sb[:, ko, g * 256:(g + 1) * 256],
                        start=(ko == 0), stop=(ko == KO - 1))
                nc.vector.tensor_tensor(
                    out=vtot_sb.rearrange("p (g i a) -> p g i a",
                                          g=2, a=65)[:, g, :, 0:64],
                    in0=pvg[0:1, 0:256].rearrange("p (i a) -> p i a", a=64),
                    in1=bv1k_sb[:, g * 256:(g + 1) * 256].rearrange(
                        "p (i a) -> p i a", a=64),
                    op=OP.add)
            nc.gpsimd.partition_broadcast(out_ap=vtb[:, :], in_ap=vtot_sb[0:1, :])

            # ---------- scores + probs (qb-centric key blocks) ----------
            # pc tiles per head-pair hp: [P, 512] = two heads x 256 query cols
            pcs = {}   # (hp, j) -> tile
            a_cur = None

            def attn_unit(j):
                """Scores/exp/mask for key-block j, all 8 heads."""
                mk = m_int if 0 < j < 8 else (m_e0 if j == 0 else m_e8)
                for h in range(H):
                    hp, sub = h // 2, h % 2
                    hr = slice(64 * sub, 64 * sub + 64)
                    sc = psS.tile([P, 256], f32, tag="s")
                    nc.tensor.matmul(
                        sc, lhsT=k_pad[hp][hr, 128 * j:128 * j + 128],
                        rhs=q_pad[hp][hr, 128 * j:128 * j + 256],
                        start=True, stop=True)
                    pc = pcp.tile([P, 256], bf16, tag=f"pc{h}", name=f"pc{h}")
                    nc.scalar.activation(out=pc, in_=sc, func=AF.Exp, scale=SCALE)
                    nc.vector.tensor_tensor(out=pc, in0=pc, in1=mk, op=OP.mult)
                    pcs[(h, j)] = pc

            def attn_av(qb):
                """AV + normalize for query block qb -> a_tok."""
                a_tok = atp.tile([P, D], f32, tag="at")
                for g in range(2):  # head group: heads 4g..4g+3
                    pav = psV.tile([P, 260], f32, tag="av")
                    nlo = ntri_lo_e if qb == 0 else ntri_lo
                    nup = ntri_up_e if qb == 7 else ntri_up
                    nc.tensor.matmul(pav, lhsT=nlo,
                                     rhs=va[:, qb, g * 260:(g + 1) * 260],
                                     start=True, stop=False)
                    nc.tensor.matmul(pav, lhsT=nup,
                                     rhs=va[:, qb + 1, g * 260:(g + 1) * 260],
                                     start=False, stop=False)
                    for i in range(4):
                        h = 4 * g + i
                        po = pav[:, i * 65:i * 65 + 65]
                        nc.tensor.matmul(
                            po, lhsT=pcs[(h, qb)][:, 128:256],
                            rhs=va[:, qb, g * 260 + i * 65:g * 260 + i * 65 + 65],
                            start=False, stop=False)
                        nc.tensor.matmul(
                            po, lhsT=pcs[(h, qb + 1)][:, 0:128],
                            rhs=va[:, qb + 1, g * 260 + i * 65:g * 260 + i * 65 + 65],
                            start=False, stop=(i == 3))
                    rc4 = small.tile([P, 4], f32, tag="rc")
                    nc.vector.tensor_scalar_add(
                        out=rc4.rearrange("p (i o) -> p i o", o=1),
                        in0=pav.rearrange("p (i a) -> p i a", a=65)[:, :, 64:65],
                        scalar1=float(S))
                    nc.vector.reciprocal(out=rc4, in_=rc4)
                    asl = a_tok[:, g * 256:(g + 1) * 256]
                    nc.vector.tensor_tensor(
                        out=asl.rearrange("p (i a) -> p i a", a=64),
                        in0=pav.rearrange("p (i a) -> p i a", a=65)[:, :, 0:64],
                        in1=vtb[:, g * 260:(g + 1) * 260].rearrange(
                            "p (i a) -> p i a", a=65)[:, :, 0:64],
                        op=OP.add)
                    nc.vector.tensor_tensor(
                        out=asl.rearrange("p (i a) -> p i a", a=64),
                        in0=asl.rearrange("p (i a) -> p i a", a=64),
                        in1=rep_last(rc4, 64), op=OP.mult)
                return a_tok

            def attn_ln(qb, a_tok):
                """LN1 -> xn (kept for residual) -> x1T feature-major."""
                xn = xnp.tile([P, D], bf16, tag=f"xn{qb}", name=f"xnt{qb}")
                layer_norm_to(a_tok, xn)
                pt = psA.tile([P, 512], bf16, tag="pt", bufs=1)
                for dc in range(KO):
                    nc.tensor.transpose(pt[:, dc * P:(dc + 1) * P],
                                        xn[:, dc * P:(dc + 1) * P], ident)
                nc.scalar.copy(
                    out=x1T[:, :, qb * P:(qb + 1) * P],
                    in_=pt.rearrange("p (ko t) -> p ko t", t=P))
                return xn

            if STAGE <= 2:
                continue
            xns = [None] * 8
            atoks = [None] * 8
            attn_unit(0)
            if STAGE >= 4:
                attn_unit(1)
                if STAGE >= 5:
                    atoks[0] = attn_av(0)
                for j in range(2, 9):
                    attn_unit(j)
                    if STAGE >= 5:
                        atoks[j - 1] = attn_av(j - 1)
                        xns[j - 2] = attn_ln(j - 2, atoks[j - 2])
                if STAGE == 5:
                    xns[7] = attn_ln(7, atoks[7])
            if STAGE <= 5:
                continue

            # ---------- FFN + residual + LN2 ----------
            hts = [htp.tile([P, 512], bf16, tag=f"h{hc}", name=f"h{hc}")
                   for hc in range(HC)]
            pend = None

            def emit_xpose(tb, xo):
                pt = psA.tile([P, 512], bf16, tag="pt", bufs=1)
                for dc in range(KO):
                    nc.tensor.transpose(pt[:, dc * P:(dc + 1) * P],
                                        xo[:, dc * P:(dc + 1) * P], ident)
                nc.scalar.copy(
                    out=xN[:, :, 64 + tb * P:64 + (tb + 1) * P],
                    in_=pt.rearrange("p (ko t) -> p ko t", t=P))

            def ffn_fc1(half):
                qs = slice(half * 512, (half + 1) * 512)
                for hc in range(HC):
                    ph = psA.tile([P, 512], f32, tag="pj")
                    for ko in range(KO):
                        nc.tensor.matmul(
                            ph, lhsT=fc1_sb[:, ko, hc * P:(hc + 1) * P],
                            rhs=x1T[:, ko, qs],
                            start=(ko == 0), stop=(ko == KO - 1))
                    nc.scalar.activation(out=hts[hc], in_=ph, func=AF.Relu,
                                         bias=fc1b_sb[:, hc:hc + 1])

            for half in range(2):
                ffn_fc1(half)
                if half == 0:
                    # qb7's LN1/transposes land while fc1-half0 runs on PE
                    xns[7] = attn_ln(7, atoks[7])
                for tb2 in range(4):
                    tb = half * 4 + tb2
                    pf = psA.tile([P, 512], f32, tag="pj")
                    # fc2 bias via K=1 matmul (start=True zeroes the bank)
                    nc.tensor.matmul(pf, lhsT=ones1[0:1, :], rhs=fc2b_sb[0:1, :],
                                     start=True, stop=False)
                    for hc in range(HC):
                        nc.tensor.matmul(
                            pf, lhsT=hts[hc][:, tb2 * P:(tb2 + 1) * P],
                            rhs=fc2_sb[:, hc, :],
                            start=False, stop=False)
                    # residual via identity matmul
                    nc.tensor.matmul(pf, lhsT=ident, rhs=xns[tb],
                                     start=False, stop=True)
                    xo = xxp.tile([P, D], bf16, tag="xo")
                    layer_norm_to(pf, xo)
                    if last:
                        # final projection partial: red[p, c, tb]
                        # (reduce split DVE/Act to avoid a DVE backlog tail)
                        for c in range(C):
                            sc_t = scr if c % 2 == 0 else scr2
                            nc.vector.tensor_tensor(
                                out=sc_t, in0=xo, in1=ow_sb[:, c, tb, :],
                                op=OP.mult)
                            if c % 2 == 0:
                                nc.vector.reduce_sum(
                                    out=red[:, c, tb:tb + 1], in_=sc_t,
                                    axis=mybir.AxisListType.X)
                            else:
                                nc.scalar.activation(
                                    out=sc_t, in_=sc_t, func=AF.Identity,
                                    accum_out=red[:, c, tb:tb + 1])
                    else:
                        # defer transpose one tb so PE never waits on LN2
                        if pend is not None:
                            emit_xpose(*pend)
                        pend = (tb, xo)
            if pend is not None:
                emit_xpose(*pend)
            xT = xN

        # ---------- final cross-partition reduce ----------
        if STAGE <= 8:
            nc.vector.memset(red[:, :, :], 0.0)
        pout = psS.tile([P, 256], f32, tag="s")
        nc.tensor.matmul(pout[0:1, 0:C * 8], lhsT=ones_col[:, 0:1],
                         rhs=red.rearrange("p c t -> p (c t)"),
                         start=True, stop=True)
        nc.vector.reduce_sum(
            out=osb, in_=pout[0:1, 0:C * 8].rearrange("p (c t) -> p c t", t=8),
            axis=mybir.AxisListType.X)
        nc.sync.dma_start(out_d[:], osb)

    nc.compile()
    return nc


def _prep(inputs):
    """Host-side input prep shared across cores. Returns (common, per_core, affine)."""
    import ml_dtypes
    bf = ml_dtypes.bfloat16

    emb = np.asarray(inputs['emb'], dtype=np.float32)
    idx = np.asarray(inputs['inputs'])
    pos = np.arange(S, dtype=np.float32)[:, None]
    div = np.exp(-np.log(10000.0) * np.arange(0, D, 2, dtype=np.float32) / D)
    ang = pos * div
    pe = np.zeros((S, D), dtype=np.float32)
    pe[:, 0::2] = np.sin(ang)
    pe[:, 1::2] = np.cos(ang)
    x0 = emb[idx] + pe[None]  # [B, S, D]

    # masks (bf16): interior M[p,c] = (p < c) & (p >= c-128) for key block
    # B_j vs query cols [128j-128, 128j+128)
    p_ = np.arange(P)[:, None]
    c_ = np.arange(256)[None, :]
    m_int = ((p_ < c_) & (p_ >= c_ - 128)).astype(np.float32)
    m_e0 = m_int * (p_ >= 64)     # j=0: keys [-64, 64), first 64 partitions fake
    m_e8 = m_int * (p_ < 64)      # j=8: keys [960, 1088), last 64 fake
    c128 = np.arange(128)[None, :]
    tri_lo = (p_ >= c128).astype(np.float32)
    tri_up = (p_ < c128).astype(np.float32)
    ntri_lo = -tri_lo
    ntri_lo_e = -(tri_lo * (p_ >= 64))
    ntri_up = -tri_up
    ntri_up_e = -(tri_up * (p_ < 64))
    mask = np.concatenate(
        [m_int, m_e0, m_e8, ntri_lo, ntri_lo_e, ntri_up, ntri_up_e],
        axis=1).astype(bf)

    ln_g = np.asarray(inputs['ln_g'], dtype=np.float32)
    ln_b = np.asarray(inputs['ln_b'], dtype=np.float32)
    affine = not (np.all(ln_g == 1.0) and np.all(ln_b == 0.0))

    def wmaj(wT, ko):  # [D, N] feature-major -> [P, ko, N]
        N = wT.shape[1]
        return np.ascontiguousarray(
            wT.reshape(ko, P, N).transpose(1, 0, 2)).astype(bf)

    out_w = np.asarray(inputs['out_w'], dtype=np.float32)
    owT = np.ascontiguousarray(
        out_w.reshape(C, 8, P, D).transpose(2, 0, 1, 3)).astype(bf)  # [P,C,8,D]

    bq = np.asarray(inputs['bq'], np.float32)
    bk = np.asarray(inputs['bk'], np.float32)
    bv = np.asarray(inputs['bv'], np.float32)
    fc1b = np.asarray(inputs['fc1_b'], np.float32)

    common = {
        'wqT': wmaj(np.asarray(inputs['wq'], np.float32).T, KO),
        'wkT': wmaj(np.asarray(inputs['wk'], np.float32).T, KO),
        'wvT': wmaj(np.asarray(inputs['wv'], np.float32).T, KO),
        'fc1T': wmaj(np.asarray(inputs['fc1_w'], np.float32).T, KO),
        'fc2T': wmaj(np.asarray(inputs['fc2_w'], np.float32).T, HC),
        'bq': np.ascontiguousarray(bq.reshape(KO, P).T),
        'bk': np.ascontiguousarray(bk.reshape(KO, P).T),
        'bv': np.ascontiguousarray(bv),
        'bv1k': np.ascontiguousarray(bv[None, :] * float(S)),
        'fc1b': np.ascontiguousarray(fc1b.reshape(HC, P).T),
        'fc2b': np.ascontiguousarray(
            np.asarray(inputs['fc2_b'], np.float32)[None, :]).astype(bf),
        'mask': mask,
        'owT': owT,
    }
    if affine:
        common['lng'] = np.ascontiguousarray(ln_g)
        common['lnb'] = np.ascontiguousarray(ln_b)
    per_core = []
    for b in range(B):
        xp = np.zeros((D, XW), dtype=np.float32)
        xp[:, 64:64 + S] = x0[b].T
        per_core.append({'xT': np.ascontiguousarray(
            xp.reshape(KO, P, XW).transpose(1, 0, 2)).astype(bf)})
    return common, per_core, affine


def kernel(**inputs):
    global LAST_EXEC_NS, LAST_RESULTS
    from concourse.bass_utils import run_bass_kernel_spmd

    common, per_core, affine = _prep(inputs)
    if affine not in _CACHE:
        _CACHE[affine] = _build(affine)
    nc = _CACHE[affine]

    in_maps = [dict(common, **pc) for pc in per_core]
    res = run_bass_kernel_spmd(nc, in_maps, list(range(B)), trace=TRACE)
    LAST_EXEC_NS = res.exec_time_ns
    LAST_RESULTS = res
    out = np.stack([res.results[b]["out"][0] for b in range(B)], axis=0)
    out = out + np.asarray(inputs['out_b'], np.float32)[None, :]
    return out.astype(np.float32)


# revision 35
# speedup vs baseline: 1.8280x; 1.0134x over previous
"""Trainium2 Bass kernel for nn_LocalModel (6-encoder local-attention transformer).

Sharding: data-parallel over batch - B=8 batch elements, one per NeuronCore.
Each core runs the full 6-layer encoder stack + final projection for its
batch element entirely on-chip (all weights resident in SBUF in bf16),
returning a [6]-vector; the host gathers them into the [8, 6] output.

Attention uses the zero-masked-softmax identity: with out-of-window scores
set to 0 (not -inf), softmax over the full sequence satisfies
    out_i = (sum_{j in W} (e^{s_ij} - 1) v_j + sum_all v_j)
          / (sum_{j in W} (e^{s_ij} - 1) + S)
The banded scores are computed qb-centric: key blocks B_j = [128j-64,
128j+64) (tokens padded by 64 zeros each side) against query cols
[128j-128, 128j+128), giving uniform triangular masks. The "-1" term is
folded into the PSUM accumulation via negative-mask matmuls against a
64-shifted V copy (va_shift), so the DVE only does exp-mask multiply.
"""
import sys
import numpy as np

sys.path.insert(0, "/opt/trn_rl_repo")

B, S, D = 8, 1024, 512
H, Dh, W = 8, 64, 64
HD = 2048           # ffn hidden
C = 6               # classes
ENC = 6
EPS = 1e-5
P = 128
KO = D // P         # 4
HC = HD // P        # 16
SCALE = Dh ** -0.5
XW = 64 + S + 64    # padded token width for x / k tiles (1152)
QW = 128 + S + 128  # padded token width for q tiles (1280)

_CACHE = {}
LAST_EXEC_NS = None
LAST_RESULTS = None
TRACE = False


def _build(affine: bool):
    import os
    STAGE = int(os.environ.get("KSTAGE", "9"))
    import concourse.bass as bass
    import concourse.tile as tile
    from concourse import bacc, mybir
    from concourse.masks import make_identity

    f32 = mybir.dt.float32
    bf16 = mybir.dt.bfloat16
    f16 = mybir.dt.float16
    AF = mybir.ActivationFunctionType
    OP = mybir.AluOpType

    nc = bacc.Bacc()
    d = {}
    d['xT'] = nc.declare_dram_parameter("xT", [P, KO, XW], bf16, isOutput=False)
    for w in ("wqT", "wkT", "wvT"):
        d[w] = nc.declare_dram_parameter(w, [P, KO, D], bf16, isOutput=False)
    d['fc1T'] = nc.declare_dram_parameter("fc1T", [P, KO, HD], bf16, isOutput=False)
    d['fc2T'] = nc.declare_dram_parameter("fc2T", [P, HC, D], bf16, isOutput=False)
    d['owT'] = nc.declare_dram_parameter("owT", [P, C, 8, D], bf16, isOutput=False)
    d['bq'] = nc.declare_dram_parameter("bq", [P, KO], f32, isOutput=False)
    d['bk'] = nc.declare_dram_parameter("bk", [P, KO], f32, isOutput=False)
    d['bv'] = nc.declare_dram_parameter("bv", [D], f32, isOutput=False)
    d['bv1k'] = nc.declare_dram_parameter("bv1k", [1, D], f32, isOutput=False)
    d['fc1b'] = nc.declare_dram_parameter("fc1b", [P, HC], f32, isOutput=False)
    d['fc2b'] = nc.declare_dram_parameter("fc2b", [1, D], bf16, isOutput=False)
    # masks: [m_int 256 | m_e0 256 | m_e8 256 | ntri_lo 128 | ntri_lo_e 128
    #         | ntri_up 128 | ntri_up_e 128]  (bf16)
    d['mask'] = nc.declare_dram_parameter("mask", [P, 1280], f16, isOutput=False)
    if affine:
        d['lng'] = nc.declare_dram_parameter("lng", [D], f32, isOutput=False)
        d['lnb'] = nc.declare_dram_parameter("lnb", [D], f32, isOutput=False)
    out_d = nc.declare_dram_parameter("out", [1, C], f32, isOutput=True)

    def bcast_ap(dram_h, parts=P):
        # replicate a [N] dram vector across `parts` partitions
        a = dram_h[:]
        return bass.AP(tensor=a.tensor, offset=a.offset,
                       ap=[[0, parts]] + [list(x) for x in a.ap])

    def rep_mid(ap2d, reps):
        # [P, N] -> [P, reps, N] with stride-0 middle axis
        return bass.AP(tensor=ap2d.tensor, offset=ap2d.offset,
                       ap=[list(ap2d.ap[0]), [0, reps], list(ap2d.ap[1])])

    def rep_last(ap2d, reps):
        # [P, N] -> [P, N, reps] with stride-0 last axis
        return bass.AP(tensor=ap2d.tensor, offset=ap2d.offset,
                       ap=[list(ap2d.ap[0]), list(ap2d.ap[1]), [0, reps]])

    from contextlib import ExitStack
    with tile.TileContext(nc) as tc, ExitStack() as ctx:
        wpool = ctx.enter_context(tc.tile_pool(name="wpool", bufs=1))
        bigx = ctx.enter_context(tc.tile_pool(name="bigx", bufs=1))
        qkp = ctx.enter_context(tc.tile_pool(name="qkp", bufs=1))
        vap = ctx.enter_context(tc.tile_pool(name="vap", bufs=1))
        pcp = ctx.enter_context(tc.tile_pool(name="pcp", bufs=4))
        atp = ctx.enter_context(tc.tile_pool(name="atp", bufs=2))
        xnp = ctx.enter_context(tc.tile_pool(name="xnp", bufs=1))
        htp = ctx.enter_context(tc.tile_pool(name="htp", bufs=1))
        xxp = ctx.enter_context(tc.tile_pool(name="xxp", bufs=2))
        tmp = ctx.enter_context(tc.tile_pool(name="tmp", bufs=3))
        small = ctx.enter_context(tc.tile_pool(name="small", bufs=4))
        psA = ctx.enter_context(tc.tile_pool(name="psA", bufs=2, space="PSUM"))
        psS = ctx.enter_context(tc.tile_pool(name="psS", bufs=3, space="PSUM"))
        psV = ctx.enter_context(tc.tile_pool(name="psV", bufs=2, space="PSUM"))

        # ---- persistent loads (host pre-arranged; all contiguous DMAs) ----
        # xA first so layer-0 V can start immediately; ow last (layer-6 only)
        xA = bigx.tile([P, KO, XW], bf16, tag="xA")
        nc.sync.dma_start(xA, d['xT'][:])
        wq_sb = wpool.tile([P, KO, D], bf16, tag="wq")
        wk_sb = wpool.tile([P, KO, D], bf16, tag="wk")
        wv_sb = wpool.tile([P, KO, D], bf16, tag="wv")
        fc1_sb = wpool.tile([P, KO, HD], bf16, tag="fc1")
        fc2_sb = wpool.tile([P, HC, D], bf16, tag="fc2")
        for sb, key in ((wv_sb, 'wvT'), (wq_sb, 'wqT'), (wk_sb, 'wkT'),
                        (fc1_sb, 'fc1T'), (fc2_sb, 'fc2T')):
            nc.sync.dma_start(sb, d[key][:])
        bq_sb = wpool.tile([P, KO], f32, tag="bq")
        bk_sb = wpool.tile([P, KO], f32, tag="bk")
        nc.sync.dma_start(bq_sb, d['bq'][:])
        nc.sync.dma_start(bk_sb, d['bk'][:])
        bv_bc = wpool.tile([P, D], f32, tag="bv")
        nc.gpsimd.dma_start(out=bv_bc, in_=bcast_ap(d['bv']))
        bv1k_sb = wpool.tile([1, D], f32, tag="bv1k")
        nc.sync.dma_start(bv1k_sb, d['bv1k'][:])
        fc1b_sb = wpool.tile([P, HC], f32, tag="fc1b")
        nc.sync.dma_start(fc1b_sb, d['fc1b'][:])
        fc2b_sb = wpool.tile([1, D], bf16, tag="fc2b")
        nc.sync.dma_start(fc2b_sb, d['fc2b'][:])
        mask_sb = wpool.tile([P, 1280], f16, tag="mask")
        nc.sync.dma_start(mask_sb, d['mask'][:])
        ow_sb = wpool.tile([P, C, 8, D], bf16, tag="ow")
        nc.gpsimd.dma_start(out=ow_sb, in_=d['owT'][:])
        if affine:
            g_bc = wpool.tile([P, D], f32, tag="g")
            b_bc = wpool.tile([P, D], f32, tag="b")
            nc.gpsimd.dma_start(out=g_bc, in_=bcast_ap(d['lng']))
            nc.gpsimd.dma_start(out=b_bc, in_=bcast_ap(d['lnb']))

        ident = wpool.tile([P, P], bf16, tag="id")
        make_identity(nc, ident)
        ones_col = wpool.tile([P, 1], f32, tag="onc")
        nc.vector.memset(ones_col, 1.0)
        ones1 = wpool.tile([1, P], bf16, tag="on1")
        nc.vector.memset(ones1, 1.0)
        eps_sb = wpool.tile([P, 1], f32, tag="eps")
        nc.vector.memset(eps_sb, EPS)
        # V-totals row [1, 2*(4*65)]; ones-slots hold S (set once)
        vtot_sb = wpool.tile([1, 520], f32, tag="vtot")
        nc.vector.memset(
            vtot_sb.rearrange("p (g i a) -> p g i a", g=2, a=65)[:, :, :, 64:65],
            float(S))
        vtb = wpool.tile([P, 520], f32, tag="vtb")
        red = wpool.tile([P, C, 8], f32, tag="red")
        osb = wpool.tile([1, C], f32, tag="osb")
        scr = wpool.tile([P, D], bf16, tag="scr")
        scr2 = wpool.tile([P, D], bf16, tag="scr2")

        # mask views
        m_int = mask_sb[:, 0:256]
        m_e0 = mask_sb[:, 256:512]
        m_e8 = mask_sb[:, 512:768]
        ntri_lo = mask_sb[:, 768:896]
        ntri_lo_e = mask_sb[:, 896:1024]
        ntri_up = mask_sb[:, 1024:1152]
        ntri_up_e = mask_sb[:, 1152:1280]

        # x ping-pong tiles (padded, feature-major)
        xB = bigx.tile([P, KO, XW], bf16, tag="xB")
        nc.gpsimd.memset(xB[:, :, 0:64], 0.0)
        nc.gpsimd.memset(xB[:, :, 64 + S:XW], 0.0)
        x1T = bigx.tile([P, KO, S], bf16, tag="x1T")

        q_pad = [qkp.tile([P, QW], bf16, tag=f"q{mc}", name=f"q{mc}")
                 for mc in range(KO)]
        k_pad = [qkp.tile([P, XW], bf16, tag=f"k{mc}", name=f"k{mc}")
                 for mc in range(KO)]
        for mc in range(KO):
            nc.gpsimd.memset(q_pad[mc][:, 0:128], 0.0)
            nc.gpsimd.memset(q_pad[mc][:, 128 + S:QW], 0.0)
            nc.gpsimd.memset(k_pad[mc][:, 0:64], 0.0)
            nc.gpsimd.memset(k_pad[mc][:, 64 + S:XW], 0.0)
        va = vap.tile([P, 9, 520], f16, tag="va")
        nc.vector.memset(
            va.rearrange("p s (i a) -> p s i a", a=65)[:, :, :, 64:65], 1.0)

        def layer_norm_to(src_ap, out_tile):
            """LayerNorm src [P,512] -> out_tile [P,512]."""
            st = small.tile([P, 6], f32, tag="st")
            mv = small.tile([P, 2], f32, tag="mv")
            nc.vector.bn_stats(out=st, in_=src_ap)
            nc.vector.bn_aggr(out=mv, in_=st)
            rstd = small.tile([P, 1], f32, tag="rs")
            nc.scalar.activation(out=rstd, in_=mv[:, 1:2], func=AF.Sqrt,
                                 bias=eps_sb[:, 0:1])
            nc.vector.reciprocal(out=rstd, in_=rstd)
            nc.vector.tensor_scalar(out=out_tile, in0=src_ap,
                                    scalar1=mv[:, 0:1], scalar2=rstd,
                                    op0=OP.subtract, op1=OP.mult)
            if affine:
                nc.vector.tensor_tensor(out=out_tile, in0=out_tile, in1=g_bc,
                                        op=OP.mult)
                nc.vector.tensor_tensor(out=out_tile, in0=out_tile, in1=b_bc,
                                        op=OP.add)

        xT = xA
        for L in range(ENC):
            xN = xB if (L % 2 == 0) else xA
            last = (L == ENC - 1)

            # ---------- V projection, 64-shifted token blocks ----------
            # xs reduce for V-totals is split per-ko and interleaved so the
            # DVE never blocks the V bias-adds for long.
            xs32 = small.tile([P, KO], f32, tag="xs")
            for stb in range(9):
                pv = psA.tile([P, 512], f32, tag="pj")
                for ko in range(KO):
                    nc.tensor.matmul(
                        pv, lhsT=xT[:, ko, 128 * stb:128 * stb + 128],
                        rhs=wv_sb[:, ko, :],
                        start=(ko == 0), stop=(ko == KO - 1))
                nc.vector.tensor_tensor(
                    out=va.rearrange("p s (g i a) -> p s g i a",
                                     g=2, a=65)[:, stb, :, :, 0:64],
                    in0=pv.rearrange("p (g i a) -> p g i a", g=2, a=64),
                    in1=bv_bc.rearrange("p (g i a) -> p g i a", g=2, a=64),
                    op=OP.add)
                if stb < KO:
                    nc.vector.reduce_sum(out=xs32[:, stb:stb + 1],
                                         in_=xT[:, stb:stb + 1, :],
                                         axis=mybir.AxisListType.X)

            # ---------- Q/K projections (feature-major, bf16, padded) ----------
            for mc in range(KO if STAGE >= 2 else 0):
                for half in range(2):
                    cs = slice(64 + half * 512, 64 + (half + 1) * 512)
                    pq = psA.tile([P, 512], f32, tag="pj")
                    for ko in range(KO):
                        nc.tensor.matmul(
                            pq, lhsT=wq_sb[:, ko, mc * P:(mc + 1) * P],
                            rhs=xT[:, ko, cs],
                            start=(ko == 0), stop=(ko == KO - 1))
                    nc.scalar.activation(
                        out=q_pad[mc][:, 128 + half * 512:128 + (half + 1) * 512],
                        in_=pq, func=AF.Identity, bias=bq_sb[:, mc:mc + 1])
                    pk = psA.tile([P, 512], f32, tag="pj")
                    for ko in range(KO):
                        nc.tensor.matmul(
                            pk, lhsT=wk_sb[:, ko, mc * P:(mc + 1) * P],
                            rhs=xT[:, ko, cs],
                            start=(ko == 0), stop=(ko == KO - 1))
                    nc.scalar.activation(
                        out=k_pad[mc][:, cs],
                        in_=pk, func=AF.Identity, bias=bk_sb[:, mc:mc + 1])

            # ---------- V totals: (sum_t x) @ wv + S*bv (f32) ----------
            xsr = small.tile([P, KO], bf16, tag="xsr")
            nc.scalar.copy(out=xsr, in_=xs32)
            pvt = psA.tile([P, 512], f32, tag="pj")
            for ko in range(KO):
                nc.tensor.matmul(
                    pvt[0:1, :], lhsT=xsr[:, ko:ko + 1],
                    rhs=wv_sb[:, ko, :],
                    start=(ko == 0), stop=(ko == KO - 1))
            nc.vector.tensor_tensor(
                out=vtot_sb.rearrange("p (g i a) -> p g i a",
                                      g=2, a=65)[:, :, :, 0:64],
                in0=pvt[0:1, :].rearrange("p (g i a) -> p g i a", g=2, a=64),
                in1=bv1k_sb.rearrange("p (g i a) -> p g i a", g=2, a=64),
                op=OP.add)
            nc.gpsimd.partition_broadcast(out_ap=vtb[:, :], in_ap=vtot_sb[0:1, :])

            # ---------- scores + probs (qb-centric key blocks) ----------
            # pc tiles per head-pair hp: [P, 512] = two heads x 256 query cols
            pcs = {}   # (hp, j) -> tile
            a_cur = None

            def attn_unit(j):
                """Scores/exp/mask for key-block j, all 8 heads."""
                mk = m_int if 0 < j < 8 else (m_e0 if j == 0 else m_e8)
                for h in range(H):
                    hp, sub = h // 2, h % 2
                    hr = slice(64 * sub, 64 * sub + 64)
                    sc = psS.tile([P, 256], f32, tag="s")
                    nc.tensor.matmul(
                        sc, lhsT=k_pad[hp][hr, 128 * j:128 * j + 128],
                        rhs=q_pad[hp][hr, 128 * j:128 * j + 256],
                        start=True, stop=True)
                    pc = pcp.tile([P, 256], f16, tag=f"pc{h}", name=f"pc{h}")
                    nc.scalar.activation(out=pc, in_=sc, func=AF.Exp, scale=SCALE)
                    nc.vector.tensor_tensor(out=pc, in0=pc, in1=mk, op=OP.mult)
                    pcs[(h, j)] = pc

            def attn_av(qb):
                """AV + normalize for query block qb -> a_tok."""
                a_tok = atp.tile([P, D], f32, tag="at")
                for g in range(2):  # head group: heads 4g..4g+3
                    pav = psV.tile([P, 260], f32, tag="av")
                    nlo = ntri_lo_e if qb == 0 else ntri_lo
                    nup = ntri_up_e if qb == 7 else ntri_up
                    nc.tensor.matmul(pav, lhsT=nlo,
                                     rhs=va[:, qb, g * 260:(g + 1) * 260],
                                     start=True, stop=False)
                    nc.tensor.matmul(pav, lhsT=nup,
                                     rhs=va[:, qb + 1, g * 260:(g + 1) * 260],
                                     start=False, stop=False)
                    for i in range(4):
                        h = 4 * g + i
                        po = pav[:, i * 65:i * 65 + 65]
                        nc.tensor.matmul(
                            po, lhsT=pcs[(h, qb)][:, 128:256],
                            rhs=va[:, qb, g * 260 + i * 65:g * 260 + i * 65 + 65],
                            start=False, stop=False)
                        nc.tensor.matmul(
                            po, lhsT=pcs[(h, qb + 1)][:, 0:128],
                            rhs=va[:, qb + 1, g * 260 + i * 65:g * 260 + i * 65 + 65],
                            start=False, stop=(i == 3))
                    rc4 = small.tile([P, 4], f32, tag="rc")
                    nc.vector.tensor_scalar_add(
                        out=rc4.rearrange("p (i o) -> p i o", o=1),
                        in0=pav.rearrange("p (i a) -> p i a", a=65)[:, :, 64:65],
                        scalar1=float(S))
                    nc.vector.reciprocal(out=rc4, in_=rc4)
                    asl = a_tok[:, g * 256:(g + 1) * 256]
                    nc.vector.tensor_tensor(
                        out=asl.rearrange("p (i a) -> p i a", a=64),
                        in0=pav.rearrange("p (i a) -> p i a", a=65)[:, :, 0:64],
                        in1=vtb[:, g * 260:(g + 1) * 260].rearrange(
                            "p (i a) -> p i a", a=65)[:, :, 0:64],
                        op=OP.add)
                    nc.vector.tensor_tensor(
                        out=asl.rearrange("p (i a) -> p i a", a=64),
                        in0=asl.rearrange("p (i a) -> p i a", a=64),
                        in1=rep_last(rc4, 64), op=OP.mult)
                return a_tok

            def attn_ln(qb, a_tok):
                """LN1 -> xn (kept for residual) -> x1T feature-major."""
                xn = xnp.tile([P, D], bf16, tag=f"xn{qb}", name=f"xnt{qb}")
                layer_norm_to(a_tok, xn)
                pt = psA.tile([P, 512], bf16, tag="pt", bufs=1)
                for dc in range(KO):
                    nc.tensor.transpose(pt[:, dc * P:(dc + 1) * P],
                                        xn[:, dc * P:(dc + 1) * P], ident)
                nc.scalar.copy(
                    out=x1T[:, :, qb * P:(qb + 1) * P],
                    in_=pt.rearrange("p (ko t) -> p ko t", t=P))
                return xn

            if STAGE <= 2:
                continue
            xns = [None] * 8
            atoks = [None] * 8
            attn_unit(0)
            if STAGE >= 4:
                attn_unit(1)
                if STAGE >= 5:
                    atoks[0] = attn_av(0)
                for j in range(2, 9):
                    attn_unit(j)
                    if STAGE >= 5:
                        atoks[j - 1] = attn_av(j - 1)
                        xns[j - 2] = attn_ln(j - 2, atoks[j - 2])
                if STAGE == 5:
                    xns[7] = attn_ln(7, atoks[7])
            if STAGE <= 5:
                continue

            # ---------- FFN + residual + LN2 ----------
            hts = [htp.tile([P, 512], bf16, tag=f"h{hc}", name=f"h{hc}")
                   for hc in range(HC)]
            pend = None

            def emit_xpose(tb, xo):
                pt = psA.tile([P, 512], bf16, tag="pt", bufs=1)
                for dc in range(KO):
                    nc.tensor.transpose(pt[:, dc * P:(dc + 1) * P],
                                        xo[:, dc * P:(dc + 1) * P], ident)
                nc.scalar.copy(
                    out=xN[:, :, 64 + tb * P:64 + (tb + 1) * P],
                    in_=pt.rearrange("p (ko t) -> p ko t", t=P))

            def ffn_fc1(half):
                qs = slice(half * 512, (half + 1) * 512)
                for hc in range(HC):
                    ph = psA.tile([P, 512], f32, tag="pj")
                    for ko in range(KO):
                        nc.tensor.matmul(
                            ph, lhsT=fc1_sb[:, ko, hc * P:(hc + 1) * P],
                            rhs=x1T[:, ko, qs],
                            start=(ko == 0), stop=(ko == KO - 1))
                    nc.scalar.activation(out=hts[hc], in_=ph, func=AF.Relu,
                                         bias=fc1b_sb[:, hc:hc + 1])

            for half in range(2):
                ffn_fc1(half)
                if half == 0:
                    # qb7's LN1/transposes land while fc1-half0 runs on PE
                    xns[7] = attn_ln(7, atoks[7])
                for tb2 in range(4):
                    tb = half * 4 + tb2
                    pf = psA.tile([P, 512], f32, tag="pj")
                    # fc2 bias via K=1 matmul (start=True zeroes the bank)
                    nc.tensor.matmul(pf, lhsT=ones1[0:1, :], rhs=fc2b_sb[0:1, :],
                                     start=True, stop=False)
                    for hc in range(HC):
                        nc.tensor.matmul(
                            pf, lhsT=hts[hc][:, tb2 * P:(tb2 + 1) * P],
                            rhs=fc2_sb[:, hc, :],
                            start=False, stop=False)
                    # residual via identity matmul
                    nc.tensor.matmul(pf, lhsT=ident, rhs=xns[tb],
                                     start=False, stop=True)
                    xo = xxp.tile([P, D], bf16, tag="xo")
                    layer_norm_to(pf, xo)
                    if last:
                        # final projection partial: red[p, c, tb]
                        # (reduce split DVE/Act to avoid a DVE backlog tail)
                        for c in range(C):
                            sc_t = scr if c % 2 == 0 else scr2
                            nc.vector.tensor_tensor(
                                out=sc_t, in0=xo, in1=ow_sb[:, c, tb, :],
                                op=OP.mult)
                            if c % 2 == 0:
                                nc.vector.reduce_sum(
                                    out=red[:, c, tb:tb + 1], in_=sc_t,
                                    axis=mybir.AxisListType.X)
                            else:
                                nc.scalar.activation(
                                    out=sc_t, in_=sc_t, func=AF.Identity,
                                    accum_out=red[:, c, tb:tb + 1])
                    else:
                        # defer transpose one tb so PE never waits on LN2
                        if pend is not None:
                            emit_xpose(*pend)
                        pend = (tb, xo)
            if pend is not None:
                emit_xpose(*pend)
            xT = xN

        # ---------- final cross-partition reduce ----------
        if STAGE <= 8:
            nc.vector.memset(red[:, :, :], 0.0)
        pout = psS.tile([P, 256], f32, tag="s")
        nc.tensor.matmul(pout[0:1, 0:C * 8], lhsT=ones_col[:, 0:1],
                         rhs=red.rearrange("p c t -> p (c t)"),
                         start=True, stop=True)
        nc.vector.reduce_sum(
            out=osb, in_=pout[0:1, 0:C * 8].rearrange("p (c t) -> p c t", t=8),
            axis=mybir.AxisListType.X)
        nc.sync.dma_start(out_d[:], osb)

    nc.compile()
    return nc


def _prep(inputs):
    """Host-side input prep shared across cores. Returns (common, per_core, affine)."""
    import ml_dtypes
    bf = ml_dtypes.bfloat16

    emb = np.asarray(inputs['emb'], dtype=np.float32)
    idx = np.asarray(inputs['inputs'])
    pos = np.arange(S, dtype=np.float32)[:, None]
    div = np.exp(-np.log(10000.0) * np.arange(0, D, 2, dtype=np.float32) / D)
    ang = pos * div
    pe = np.zeros((S, D), dtype=np.float32)
    pe[:, 0::2] = np.sin(ang)
    pe[:, 1::2] = np.cos(ang)
    x0 = emb[idx] + pe[None]  # [B, S, D]

    # masks (bf16): interior M[p,c] = (p < c) & (p >= c-128) for key block
    # B_j vs query cols [128j-128, 128j+128)
    p_ = np.arange(P)[:, None]
    c_ = np.arange(256)[None, :]
    m_int = ((p_ < c_) & (p_ >= c_ - 128)).astype(np.float32)
    m_e0 = m_int * (p_ >= 64)     # j=0: keys [-64, 64), first 64 partitions fake
    m_e8 = m_int * (p_ < 64)      # j=8: keys [960, 1088), last 64 fake
    c128 = np.arange(128)[None, :]
    tri_lo = (p_ >= c128).astype(np.float32)
    tri_up = (p_ < c128).astype(np.float32)
    ntri_lo = -tri_lo
    ntri_lo_e = -(tri_lo * (p_ >= 64))
    ntri_up = -tri_up
    ntri_up_e = -(tri_up * (p_ < 64))
    mask = np.concatenate(
        [m_int, m_e0, m_e8, ntri_lo, ntri_lo_e, ntri_up, ntri_up_e],
        axis=1).astype(np.float16)

    ln_g = np.asarray(inputs['ln_g'], dtype=np.float32)
    ln_b = np.asarray(inputs['ln_b'], dtype=np.float32)
    affine = not (np.all(ln_g == 1.0) and np.all(ln_b == 0.0))

    def wmaj(wT, ko):  # [D, N] feature-major -> [P, ko, N]
        N = wT.shape[1]
        return np.ascontiguousarray(
            wT.reshape(ko, P, N).transpose(1, 0, 2)).astype(bf)

    out_w = np.asarray(inputs['out_w'], dtype=np.float32)
    owT = np.ascontiguousarray(
        out_w.reshape(C, 8, P, D).transpose(2, 0, 1, 3)).astype(bf)  # [P,C,8,D]

    bq = np.asarray(inputs['bq'], np.float32)
    bk = np.asarray(inputs['bk'], np.float32)
    bv = np.asarray(inputs['bv'], np.float32)
    fc1b = np.asarray(inputs['fc1_b'], np.float32)

    common = {
        'wqT': wmaj(np.asarray(inputs['wq'], np.float32).T, KO),
        'wkT': wmaj(np.asarray(inputs['wk'], np.float32).T, KO),
        'wvT': wmaj(np.asarray(inputs['wv'], np.float32).T, KO),
        'fc1T': wmaj(np.asarray(inputs['fc1_w'], np.float32).T, KO),
        'fc2T': wmaj(np.asarray(inputs['fc2_w'], np.float32).T, HC),
        'bq': np.ascontiguousarray(bq.reshape(KO, P).T),
        'bk': np.ascontiguousarray(bk.reshape(KO, P).T),
        'bv': np.ascontiguousarray(bv),
        'bv1k': np.ascontiguousarray(bv[None, :] * float(S)),
        'fc1b': np.ascontiguousarray(fc1b.reshape(HC, P).T),
        'fc2b': np.ascontiguousarray(
            np.asarray(inputs['fc2_b'], np.float32)[None, :]).astype(bf),
        'mask': mask,
        'owT': owT,
    }
    if affine:
        common['lng'] = np.ascontiguousarray(ln_g)
        common['lnb'] = np.ascontiguousarray(ln_b)
    per_core = []
    for b in range(B):
        xp = np.zeros((D, XW), dtype=np.float32)
        xp[:, 64:64 + S] = x0[b].T
        per_core.append({'xT': np.ascontiguousarray(
            xp.reshape(KO, P, XW).transpose(1, 0, 2)).astype(bf)})
    return common, per_core, affine


def kernel(**inputs):
    global LAST_EXEC_NS, LAST_RESULTS
    from concourse.bass_utils import run_bass_kernel_spmd

    common, per_core, affine = _prep(inputs)
    if affine not in _CACHE:
        _CACHE[affine] = _build(affine)
    nc = _CACHE[affine]

    in_maps = [dict(common, **pc) for pc in per_core]
    res = run_bass_kernel_spmd(nc, in_maps, list(range(B)), trace=TRACE)
    LAST_EXEC_NS = res.exec_time_ns
    LAST_RESULTS = res
    out = np.stack([res.results[b]["out"][0] for b in range(B)], axis=0)
    out = out + np.asarray(inputs['out_b'], np.float32)[None, :]
    return out.astype(np.float32)
